# revision 48
# baseline (speedup 1.0000x reference)
"""Trainium2 Bass kernel for nn_AFW_63118839382657 (low-rank cross-modal bilinear net).

Key algebraic identity: G = (q1 outer q2) * (k1 outer k2) = (q1*k1) outer (q2*k2),
i.e. G is rank-1 per (b, t) with factors u = q1*k1, v = q2*k2 in R^32.
Everything then reduces to:
  Mqk[b]   = u[b].T @ v[b] / T                      (tiny matmuls)
  P_j[b]   = Mqk[m1] @ Mqk[m2]                      (32x32 matmuls)
  w[b]     = v[b] @ P_j[b]                          (w.T = P.T @ v.T)
  proj     = Z @ Watt,  Z[bt, k*32+o] = u[bt,k]*w[bt,o]
  out      = (proj + batt + beta) * X

fp8 version: the big matmul runs in float8e4 with MatmulPerfMode.DoubleRow.
All per-(b,t) factors and Watt are float8e4 with per-layer power-of-two scale
constants (A/SM/TT/CW/SW below).  X, projections, and outputs ride in fp16.

v2 scheduling: per-nt (512-token) PSUM tiles for projections and the big
matmul so evacuation overlaps accumulation; evac work split ACT/DVE (Pool has
no PSUM port); z-build split Pool/DVE; final muls split DVE/Pool; DMA traffic
spread across all five queues by load.  Data-parallel over batch: B=16 ->
2 per core across 8 NeuronCores, weights replicated, no collectives.
"""
import sys

for _p in ("/opt/trn_rl_repo", "/opt/pypackages"):
    if _p not in sys.path:
        sys.path.insert(0, _p)

import numpy as np
import ml_dtypes
from contextlib import ExitStack

import concourse.bass as bass
import concourse.mybir as mybir
import concourse.tile as tile
from concourse.tile import add_dep_helper
from concourse import bacc
from concourse.masks import make_identity
from concourse.bass_utils import run_bass_kernel_spmd

F32 = mybir.dt.float32
F16 = mybir.dt.float16
F8 = mybir.dt.float8e4
Copy = mybir.ActivationFunctionType.Copy
Ident = mybir.ActivationFunctionType.Identity
DR = mybir.MatmulPerfMode.DoubleRow
Mult = mybir.AluOpType.mult
Add = mybir.AluOpType.add

L, MODS, D, R, B, T = 2, 3, 512, 32, 16, 512
BETA = 0.1
NCORES = 8
BL = B // NCORES          # local batch = 2
BT = BL * T               # 1024
P = 128
KC = D // P               # 4 d-chunks (projection contraction)
KO = (R * R) // P         # 8 ko-chunks (big matmul contraction)
NPAIR = KO // 2           # 4 DoubleRow ko-chunk pairs
DT = D // P               # 4 d-tiles (big matmul output)
NT = BT // 512            # 2 bt-halves
LM = L * MODS

# per-layer fp8 scale constants (validated numerically: abs-max rel ~1.5e-3)
A = [2.0, 16.0]           # qkT scale
SM = [1.0 / 16, 1.0 / 16]  # Mqk evac scale (on top of a^4*T)
TT = [1.0 / 32, 1.0 / 16]  # P evac scale
CW = [1.0 / 16, 1.0 / 16]  # wrep evac scale
SW = 64.0                  # Watt scale
KSC = [A[li] ** 12 * T * T * SM[li] ** 2 * TT[li] * CW[li] * SW for li in range(L)]

# ---- engine assignment config (tuned against CoreSim) -----------------------
# zmul engines per (li, module): 8 ops (nt-major: b0c0..b0c3, b1c0..b1c3)
ZMUL_PAT = {
    (0, 0): "PDPPPPDP", (0, 1): "PDPPPPDP", (0, 2): "PDPPPPDP",
    (1, 0): "PDPPDPPP", (1, 1): "PDPPDPPP", (1, 2): "DPPDDPPD",
}
# big-evac engine per (li, m, (dt, nt)) half: A=ACT, D=DVE (8 chars)
BIGE_PAT = {
    (0, 0): "AAAADDAA", (0, 1): "AAAADDAA", (0, 2): "AAAADDAA",
    (1, 0): "AAAADAAA", (1, 1): "AAAADAAA", (1, 2): "AAAADAAA",
}
# final-mul engine per (li, m, (dt, nt)) half: D=DVE, P=Pool (8 chars)
FMUL_PAT = {
    (0, 0): "DDDDPPDD", (0, 1): "DDDDPPDD", (0, 2): "DDDDPPDD",
    (1, 0): "DDDDDDDD", (1, 1): "DDDDDDDD", (1, 2): "DDDDDDDD",
}
# wrep evac engines per module (2 ops, run in parallel): A/D
WREPE_PAT = "AD"
# qkT evac engines per layer (nt0, nt1): parallel halves for layer 1
QKTE_PAT = {0: "AA", 1: "AD"}
# watt load queue per lm: s=sync c=scalar g=gpsimd (DVE has no DGE port)
WATT_Q = "gcgcgc"

# urep load queue per (module, nt): 12 entries
UREP_Q = "ssssssssssss"
# ud store queue per (module, nt)
UD_Q = "ssssssssssss"
# out store queue per (m, dt): 12 entries (last module's last dt handled by fine path)
OUT_Q = {(0): "sscs", (1): "sscs", (2): "sscs"}
# fully fine-grained last module: per-(dt,nt) evac/mul engines + store queues
FINE_EVAC = "ADADADDA"
FINE_MUL = "DPDPDPPD"
FINE_STQ = "sgscsgcs"


def build_graph(nc):
    xt = nc.dram_tensor("xt", [MODS, D, BT], F16, kind="ExternalInput").ap()
    # host-repacked into SBUF layout for single-descriptor loads
    wqk = nc.dram_tensor("wqk", [P, LM, KC, P], F16, kind="ExternalInput").ap()
    bqk = nc.dram_tensor("bqk", [P, LM], F32, kind="ExternalInput").ap()
    watt = nc.dram_tensor("watt", [P, LM, KO, D], F8, kind="ExternalInput").ap()
    bout = nc.dram_tensor("bout", [P, LM, DT], F32, kind="ExternalInput").ap()
    out = nc.dram_tensor("out", [MODS, D, BT], F16, kind="ExternalOutput").ap()

    with tile.TileContext(nc) as tc, ExitStack() as ctx:
        const = ctx.enter_context(tc.tile_pool(name="const", bufs=1))
        xpool = ctx.enter_context(tc.tile_pool(name="xpool", bufs=16))
        xmp = ctx.enter_context(tc.tile_pool(name="xmp", bufs=2))
        wattp = ctx.enter_context(tc.tile_pool(name="wattp", bufs=6))
        qkp = ctx.enter_context(tc.tile_pool(name="qkp", bufs=3))
        uvp = ctx.enter_context(tc.tile_pool(name="uvp", bufs=4))
        natp = ctx.enter_context(tc.tile_pool(name="natp", bufs=3))
        mp = ctx.enter_context(tc.tile_pool(name="mp", bufs=8))
        pp_ = ctx.enter_context(tc.tile_pool(name="pp", bufs=6))
        wrp = ctx.enter_context(tc.tile_pool(name="wrp", bufs=3))
        urp = ctx.enter_context(tc.tile_pool(name="urp", bufs=3))
        zp = ctx.enter_context(tc.tile_pool(name="zp", bufs=10))
        rp = ctx.enter_context(tc.tile_pool(name="rp", bufs=3))
        op_ = ctx.enter_context(tc.tile_pool(name="op", bufs=3))
        dramp = ctx.enter_context(tc.tile_pool(name="dramp", bufs=4, space="DRAM"))

        # one PSUM pool, tag-partitioned: qk 2 banks, big 4 banks, sm 2 banks
        psp = ctx.enter_context(tc.tile_pool(name="psp", bufs=2, space="PSUM"))

        Q = {"s": nc.sync, "c": nc.scalar, "g": nc.gpsimd, "v": nc.vector}

        # identity first (Pool), head x/weight loads spread so the first
        # projection's chunks land earliest and nothing blocks ACT evacs
        ident = const.tile([P, P], F16)
        make_identity(nc, ident)
        identh = const.tile([P, 64], F16)
        make_identity(nc, identh[64:128, :])

        wqk_sb = const.tile([P, LM, KC, P], F16)
        bqk_sb = const.tile([P, LM], F32)
        bout_sb = const.tile([P, LM, DT], F32)
        nc.sync.dma_start(out=wqk_sb[:, 0, :, :], in_=wqk[:, 0])
        xt_cur = [[None] * KC for _ in range(MODS)]
        xv0 = xt[0].rearrange("(c p) bt -> p c bt", p=P)
        xm0_dmas = []
        for c in range(KC):
            xmc = xpool.tile([P, BT], F16, tag="x", name=f"xm0c{c}")
            d = (nc.sync if c % 2 == 0 else nc.gpsimd).dma_start(
                out=xmc, in_=xv0[:, c, :]
            )
            xm0_dmas.append(d)
            xt_cur[0][c] = xmc
        nc.sync.dma_start(out=bqk_sb, in_=bqk)
        # m1/m2 x tiles split in halves across queues; nosync deps keep the
        # scheduler from hoisting them ahead of module-0's critical chunks
        for m, q0, q1 in ((1, "c", "s"), (2, "g", "g")):
            xm = xmp.tile([P, KC, BT], F16, tag="xm", name=f"xm{m}")
            xv = xt[m].rearrange("(c p) bt -> p c bt", p=P)
            d0 = Q[q0].dma_start(out=xm[:, 0:2, :], in_=xv[:, 0:2, :])
            d1 = Q[q1].dma_start(out=xm[:, 2:KC, :], in_=xv[:, 2:KC, :])
            for d, dep in ((d0, xm0_dmas[1]), (d1, xm0_dmas[2 if m == 1 else 3])):
                add_dep_helper(d.ins, dep.ins, sync=False, reason="head order")
            for c in range(KC):
                xt_cur[m][c] = xm[:, c, :]
        wqk_rest = nc.sync.dma_start(
            out=wqk_sb[:, 1:MODS, :, :], in_=wqk[:, 1:MODS]
        )
        add_dep_helper(wqk_rest.ins, xm0_dmas[2].ins, sync=False,
                       reason="head order")


        # Per-layer state, keyed by layer index.
        S = {
            li: dict(Ms={}, Ps={}, uvTs=[None] * MODS, ut_dr=[None] * MODS,
                     watt_sb=[None] * MODS, wreps=[None] * MODS,
                     ureps=[None] * MODS, zTs=[None] * MODS)
            for li in range(L)
        }

        def s1(li, m):
            """Projections, u/v factors, Mqk forms for (layer, modality)."""
            st = S[li]
            lm = li * MODS + m
            qkT = qkp.tile([P, BT], F16, tag="qkT", name=f"qkT{lm}")
            uvT = uvp.tile([64, BT], F8, tag="uvT", name=f"uvT{lm}")
            ud = dramp.tile([4, NT, KO, 512], F8, tag="ut", name=f"ud{lm}")
            psqs = []
            # PE: proj(nt0) x4, proj(nt1) x4 emitted back-to-back so PE keeps
            # busy while ACT evacuates the first half.
            for nt in range(NT):
                sl = slice(nt * 512, (nt + 1) * 512)
                psq = psp.tile([P, 512], F32, tag="qk", bufs=2, name=f"psq{lm}{nt}")
                psqs.append(psq)
                for c in range(KC):
                    nc.tensor.matmul(
                        psq,
                        lhsT=wqk_sb[:, lm, c, :],
                        rhs=xt_cur[m][c][:, sl],
                        start=(c == 0),
                        stop=(c == KC - 1),
                    )
            psks = []
            last_evac = None
            for nt in range(NT):
                sl = slice(nt * 512, (nt + 1) * 512)
                if QKTE_PAT[li][nt] == "A":
                    last_evac = nc.scalar.activation(
                        out=qkT[:, sl], in_=psqs[nt], func=Ident,
                        scale=A[li], bias=bqk_sb[:, lm : lm + 1],
                    )
                else:
                    last_evac = nc.vector.tensor_scalar(
                        out=qkT[:, sl], in0=psqs[nt],
                        scalar1=A[li], scalar2=bqk_sb[:, lm : lm + 1],
                        op0=Mult, op1=Add,
                    )
                # shift k-rows to partitions 0:64 via identity matmul
                psk = psp.tile([64, 512], F32, tag="sm", bufs=2, name=f"psk{lm}{nt}")
                psks.append(psk)
                nc.tensor.matmul(
                    psk,
                    lhsT=identh[64:128, :],
                    rhs=qkT[64:128, sl],
                    tile_position=(64, 0),
                )
                nc.vector.tensor_mul(
                    out=uvT[:, sl], in0=qkT[0:64, sl], in1=psks[nt]
                )
                Q[UD_Q[lm * NT + nt]].dma_start(
                    out=ud[:, nt].rearrange("k c j -> c k j"), in_=uvT[0:32, sl]
                )
            st["uvTs"][m] = uvT
            st["ut_dr"][m] = ud

            # t-major u/v: transpose fp16 qkT chunks; evacuate only the k-half
            # to SBUF (ACT), then q(PSUM) * k(SBUF) in one DVE op
            pst = psp.tile([P, KO, P], F16, tag="sm", bufs=2, name=f"pst{lm}")
            for c8 in range(KO):
                nc.tensor.transpose(
                    pst[:, c8, :], qkT[:, c8 * P : (c8 + 1) * P], ident
                )
            t16 = natp.tile([P, KO, P], F16, tag="natt", name=f"natt{lm}")
            nc.vector.tensor_copy(out=t16, in_=pst)
            uv_nat = natp.tile([P, KO, 64], F8, tag="nat", name=f"nat{lm}")
            nc.gpsimd.tensor_mul(
                out=uv_nat, in0=t16[:, :, 0:64], in1=t16[:, :, 64:128]
            )

            forms = []
            if m in (0, 1):
                forms.append("L")
            if m in (1, 2):
                forms.append("R")
            slots = [(b, f) for b in range(BL) for f in forms]
            pm = psp.tile([32, len(slots), 32], F32, tag="sm", bufs=2, name=f"pm{lm}")
            for si, (b, f) in enumerate(slots):
                for cc in range(4):
                    ch = b * 4 + cc
                    if f == "L":
                        lhs = uv_nat[:, ch, 32:64]
                        rhs = uv_nat[:, ch, 0:32]
                    else:
                        lhs = uv_nat[:, ch, 0:32]
                        rhs = uv_nat[:, ch, 32:64]
                    nc.tensor.matmul(
                        pm[:, si, :], lhsT=lhs, rhs=rhs,
                        start=(cc == 0), stop=(cc == 3),
                    )
            msb = mp.tile([32, len(slots), 32], F8, tag="m", name=f"M{m}")
            nc.vector.tensor_scalar_mul(msb, pm, SM[li])
            for si, (b, f) in enumerate(slots):
                st["Ms"][(f, m, b)] = msb[:, si, :]
            wsb = wattp.tile([P, KO, D], F8, tag="watt", name=f"wsb{lm}")
            wdma = Q[WATT_Q[lm]].dma_start(out=wsb, in_=watt[:, lm])
            # scheduling-only edge: keep this bulky load from jumping ahead
            # of the module's own critical evacs on an idle queue
            add_dep_helper(wdma.ins, last_evac.ins, sync=False,
                           reason="pace watt load")
            st["watt_sb"][m] = wsb

        def pblock(li, js=range(MODS)):
            """Cross-modal P products; emits P tiled 4x along free dim so the
            w-matmul can write the partition-replicated wrep directly."""
            st = S[li]
            for j in js:
                for b in range(BL):
                    m1, m2 = [x for x in range(MODS) if x != j]
                    rhs4 = st["Ms"][("R", m2, b)][:, None, :].to_broadcast((32, 4, 32))
                    pps = psp.tile([64, 4, 32], F32, tag="sm", bufs=2,
                                   name=f"pps{li}{j}{b}")
                    nc.tensor.matmul(
                        pps[32:64],
                        lhsT=st["Ms"][("L", m1, b)],
                        rhs=rhs4,
                        tile_position=(0, 32),
                    )
                    ph = pp_.tile([64, 4, 32], F8, tag="p", name=f"ph{li}{j}{b}")
                    nc.vector.tensor_scalar_mul(ph[32:64], pps[32:64], TT[li])
                    st["Ps"][(j, b)] = ph

        def prep(li, m):
            """urep pair broadcast load + direct partition-replicated w (wrep)."""
            st = S[li]
            lm = li * MODS + m
            urall = urp.tile([P, NT, KO, 512], F8, tag="urep", name=f"ur{li}{m}")
            for nt in range(NT):
                src = st["ut_dr"][m][:, nt].rearrange("k c j -> k (c j)")[
                    :, None, :
                ].to_broadcast((4, 32, KO * 512))
                Q[UREP_Q[lm * NT + nt]].dma_start(
                    out=urall[:, nt].rearrange("p c j -> p (c j)"), in_=src
                )
            st["ureps"][m] = urall
            wrep = wrp.tile([P, BT], F8, tag="wrep", name=f"wrep{li}{m}")
            for b in range(BL):
                pw = psp.tile([P, 512], F32, tag="sm", bufs=2, name=f"pw{li}{m}{b}")
                nc.tensor.matmul(
                    pw,
                    lhsT=st["Ps"][(m, b)][32:64].rearrange("p a b -> p (a b)"),
                    rhs=st["uvTs"][m][32:64, b * 512 : (b + 1) * 512],
                    tile_position=(32, 0),
                )
                if WREPE_PAT[b] == "A":
                    wev = nc.scalar.activation(
                        out=wrep[:, b * 512 : (b + 1) * 512], in_=pw, func=Copy,
                        scale=CW[li],
                    )
                else:
                    wev = nc.vector.tensor_scalar_mul(
                        wrep[:, b * 512 : (b + 1) * 512], pw, CW[li]
                    )
            st["wreps"][m] = wrep
            return wev

        def zmuls(li, m):
            """Z.T pair tiles [128, 2, 512] = urep * wrep, split Pool/DVE."""
            st = S[li]
            urall = st["ureps"][m]
            zT = []
            for c2 in range(NPAIR):
                zc = zp.tile([P, 2, BT], F8, tag="zT", name=f"z{li}{m}c{c2}")
                zT.append(zc)
            # nt-major emission so big(nt0) can start while nt1 z-chunks build
            for idx in range(2 * NPAIR):
                b, c2 = idx // NPAIR, idx % NPAIR
                hs = slice(b * 512, (b + 1) * 512)
                eng = nc.gpsimd if ZMUL_PAT[(li, m)][idx] == "P" else nc.vector
                eng.tensor_mul(
                    out=zT[c2][:, :, hs],
                    in0=urall[:, b, 2 * c2 : 2 * c2 + 2, :],
                    in1=st["wreps"][m][:, None, hs].to_broadcast((P, 2, 512)),
                )
            st["zTs"][m] = zT

        def big(li, m):
            """DoubleRow fp8 matmul proj.T = Watt.T @ Z.T + residual combine.
            nt-granular psum tiles so evac overlaps accumulation."""
            st = S[li]
            lm = li * MODS + m
            zT = st["zTs"][m]
            if li == 0:
                xnew = [
                    xpool.tile([P, BT], F16, tag="x", name=f"xn{m}c{c}")
                    for c in range(KC)
                ]
            else:
                outm = out[m].rearrange("(t p) bt -> p t bt", p=P)
            for dt_i in range(DT):
                fine = li == 1 and m == 2
                res = rp.tile([P, BT], F16, tag="res")
                if li == 0:
                    tgt = xnew[dt_i]
                else:
                    tgt = op_.tile([P, BT], F16, tag="ost")
                for nt in range(NT):
                    sl = slice(nt * 512, (nt + 1) * 512)
                    pbig = psp.tile([P, 512], F32, tag="big", bufs=4, name="pbig")
                    for c2 in range(NPAIR):
                        nc.tensor.matmul(
                            pbig,
                            lhsT=st["watt_sb"][m][:, 2 * c2 : 2 * c2 + 2,
                                                  dt_i * P : (dt_i + 1) * P],
                            rhs=zT[c2][:, :, sl],
                            perf_mode=DR,
                            start=(c2 == 0),
                            stop=(c2 == NPAIR - 1),
                        )
                    hi = dt_i * NT + nt
                    ev = FINE_EVAC[hi] if fine else BIGE_PAT[(li, m)][hi]
                    if ev == "A":
                        nc.scalar.activation(
                            out=res[:, sl], in_=pbig, func=Ident,
                            scale=1.0 / KSC[li],
                            bias=bout_sb[:, lm, dt_i : dt_i + 1],
                        )
                    else:
                        nc.vector.tensor_scalar(
                            out=res[:, sl], in0=pbig,
                            scalar1=1.0 / KSC[li],
                            scalar2=bout_sb[:, lm, dt_i : dt_i + 1],
                            op0=Mult, op1=Add,
                        )
                    fm = FINE_MUL[hi] if fine else FMUL_PAT[(li, m)][hi]
                    eng = nc.vector if fm == "D" else nc.gpsimd
                    eng.tensor_mul(
                        out=tgt[:, sl], in0=res[:, sl],
                        in1=xt_cur[m][dt_i][:, sl],
                    )
                    if fine:
                        Q[FINE_STQ[hi]].dma_start(
                            out=outm[:, dt_i, sl], in_=tgt[:, sl]
                        )
                if li == 1 and not fine:
                    Q[OUT_Q[m][dt_i]].dma_start(out=outm[:, dt_i, :], in_=tgt)
            if li == 0:
                xt_cur[m] = xnew

        # ---- software-pipelined emission: layer-2 stage-1 hides under
        # ---- layer-1 big matmuls.
        for m in range(MODS):
            s1(0, m)
        pblock(0, js=(0,))
        wev00 = prep(0, 0)
        wq35 = nc.sync.dma_start(out=wqk_sb[:, MODS:, :, :], in_=wqk[:, MODS:])
        add_dep_helper(wq35.ins, wev00.ins, sync=False, reason="pace wqk l1")
        zmuls(0, 0)
        pblock(0, js=(1,))
        prep(0, 1)
        zmuls(0, 1)
        pblock(0, js=(2,))
        prep(0, 2)
        nc.gpsimd.dma_start(out=bout_sb, in_=bout)
        big(0, 0)
        s1(1, 0)
        zmuls(0, 2)
        big(0, 1)
        s1(1, 1)
        big(0, 2)
        s1(1, 2)
        pblock(1, js=(0,))
        prep(1, 0)
        zmuls(1, 0)
        pblock(1, js=(1,))
        prep(1, 1)
        zmuls(1, 1)
        pblock(1, js=(2,))
        prep(1, 2)
        big(1, 0)
        zmuls(1, 2)
        big(1, 1)
        big(1, 2)

    nc.finalize()
    return nc


_NC_CACHE = None


def _get_nc():
    global _NC_CACHE
    if _NC_CACHE is None:
        nc = bacc.Bacc("TRN2", target_bir_lowering=False, debug=False)
        _NC_CACHE = build_graph(nc)
    return _NC_CACHE


def make_in_maps(inputs):
    wqk = np.concatenate(
        [inputs["Wq1"], inputs["Wq2"], inputs["Wk1"], inputs["Wk2"]], axis=-1
    ).reshape(LM, D, 128)
    bqk_f = np.concatenate(
        [inputs["bq1"], inputs["bq2"], inputs["bk1"], inputs["bk2"]], axis=-1
    ).reshape(LM, 128).astype(np.float32)
    # pre-scale bias by the per-layer qkT scale (activation applies
    # out = in*scale + bias, so bias needs the same scale as the matmul)
    ascale = np.repeat([A[0], A[1]], MODS).astype(np.float32)[:, None]
    # repack to SBUF layouts: wqk [p, lm, c, w]; bqk [p, lm]; watt [p, lm, c, d];
    # bout [p, lm, dt]
    wqk_r = np.ascontiguousarray(
        wqk.reshape(LM, KC, P, 128).transpose(2, 0, 1, 3)
    ).astype(np.float16)
    bqk_r = np.ascontiguousarray((bqk_f * ascale).T)
    watt = np.asarray(inputs["Watt"], np.float32).reshape(LM, R * R, D)
    watt_f8 = np.clip(watt * SW, -240, 240).astype(ml_dtypes.float8_e4m3)
    watt_r = np.ascontiguousarray(
        watt_f8.reshape(LM, KO, P, D).transpose(2, 0, 1, 3)
    )
    bout = (np.asarray(inputs["batt"], np.float32) + np.float32(BETA)).reshape(
        LM, D
    )
    bout_r = np.ascontiguousarray(
        bout.reshape(LM, DT, P).transpose(2, 0, 1)
    )
    xs = [np.asarray(inputs[k], np.float32) for k in ("x_a", "x_t", "x_v")]
    in_maps = []
    for core in range(NCORES):
        sh = slice(core * BL, (core + 1) * BL)
        xts = np.stack(
            [np.ascontiguousarray(x[sh].reshape(BT, D).T) for x in xs]
        ).astype(np.float16)
        in_maps.append(
            {
                "xt": xts,
                "wqk": wqk_r,
                "bqk": bqk_r,
                "watt": watt_r,
                "bout": bout_r,
            }
        )
    return in_maps


def assemble(results):
    full = [np.empty((B, T, D), np.float32) for _ in range(MODS)]
    for core in range(NCORES):
        o = results[core]["out"]  # [MODS, D, BT] fp16
        for m in range(MODS):
            full[m][core * BL : (core + 1) * BL] = (
                o[m].T.reshape(BL, T, D).astype(np.float32)
            )
    return tuple(full)


def kernel(**inputs):
    nc = _get_nc()
    in_maps = make_in_maps(inputs)
    last_err = None
    for attempt in range(3):
        try:
            res = run_bass_kernel_spmd(nc, in_maps, core_ids=list(range(NCORES)))
            return assemble(res.results)
        except Exception as e:  # transient NRT_EXEC_UNIT_UNRECOVERABLE wedges
            last_err = e
            if attempt < 2:
                import time

                time.sleep(90)
    raise last_err


# revision 50
# speedup vs baseline: 1.0122x; 1.0122x over previous
"""Trainium2 Bass kernel for nn_AFW_63118839382657 (low-rank cross-modal bilinear net).

Key algebraic identity: G = (q1 outer q2) * (k1 outer k2) = (q1*k1) outer (q2*k2),
i.e. G is rank-1 per (b, t) with factors u = q1*k1, v = q2*k2 in R^32.
Everything then reduces to:
  Mqk[b]   = u[b].T @ v[b] / T                      (tiny matmuls)
  P_j[b]   = Mqk[m1] @ Mqk[m2]                      (32x32 matmuls)
  w[b]     = v[b] @ P_j[b]                          (w.T = P.T @ v.T)
  proj     = Z @ Watt,  Z[bt, k*32+o] = u[bt,k]*w[bt,o]
  out      = (proj + batt + beta) * X

fp8 version: the big matmul runs in float8e4 with MatmulPerfMode.DoubleRow.
All per-(b,t) factors and Watt are float8e4 with per-layer power-of-two scale
constants (A/SM/TT/CW/SW below).  X, projections, and outputs ride in fp16.

v2 scheduling: per-nt (512-token) PSUM tiles for projections and the big
matmul so evacuation overlaps accumulation; evac work split ACT/DVE (Pool has
no PSUM port); z-build split Pool/DVE; final muls split DVE/Pool; DMA traffic
spread across all five queues by load.  Data-parallel over batch: B=16 ->
2 per core across 8 NeuronCores, weights replicated, no collectives.
"""
import sys

for _p in ("/opt/trn_rl_repo", "/opt/pypackages"):
    if _p not in sys.path:
        sys.path.insert(0, _p)

import numpy as np
import ml_dtypes
from contextlib import ExitStack

import concourse.bass as bass
import concourse.mybir as mybir
import concourse.tile as tile
from concourse.tile import add_dep_helper
from concourse import bacc
from concourse.masks import make_identity
from concourse.bass_utils import run_bass_kernel_spmd

F32 = mybir.dt.float32
F16 = mybir.dt.float16
F8 = mybir.dt.float8e4
Copy = mybir.ActivationFunctionType.Copy
Ident = mybir.ActivationFunctionType.Identity
DR = mybir.MatmulPerfMode.DoubleRow
Mult = mybir.AluOpType.mult
Add = mybir.AluOpType.add

L, MODS, D, R, B, T = 2, 3, 512, 32, 16, 512
BETA = 0.1
NCORES = 8
BL = B // NCORES          # local batch = 2
BT = BL * T               # 1024
P = 128
KC = D // P               # 4 d-chunks (projection contraction)
KO = (R * R) // P         # 8 ko-chunks (big matmul contraction)
NPAIR = KO // 2           # 4 DoubleRow ko-chunk pairs
DT = D // P               # 4 d-tiles (big matmul output)
NT = BT // 512            # 2 bt-halves
LM = L * MODS

# per-layer fp8 scale constants (validated numerically: abs-max rel ~1.5e-3)
A = [2.0, 16.0]           # qkT scale
SM = [1.0 / 16, 1.0 / 16]  # Mqk evac scale (on top of a^4*T)
TT = [1.0 / 32, 1.0 / 16]  # P evac scale
CW = [1.0 / 16, 1.0 / 16]  # wrep evac scale
SW = 64.0                  # Watt scale
KSC = [A[li] ** 12 * T * T * SM[li] ** 2 * TT[li] * CW[li] * SW for li in range(L)]

# ---- engine assignment config (tuned against CoreSim) -----------------------
# zmul engines per (li, module): 8 ops (nt-major: b0c0..b0c3, b1c0..b1c3)
ZMUL_PAT = {
    (0, 0): "PDPPPPDP", (0, 1): "PDPPPPDP", (0, 2): "PDPPPPDP",
    (1, 0): "PDPPDPPP", (1, 1): "PDPPDPPP", (1, 2): "DPPDDPPD",
}
# big-evac engine per (li, m, (dt, nt)) half: A=ACT, D=DVE (8 chars)
BIGE_PAT = {
    (0, 0): "AAAADDAA", (0, 1): "AAAADDAA", (0, 2): "AAAADDAA",
    (1, 0): "AAAADAAA", (1, 1): "AAAADAAA", (1, 2): "AAAADAAA",
}
# final-mul engine per (li, m, (dt, nt)) half: D=DVE, P=Pool (8 chars)
FMUL_PAT = {
    (0, 0): "DDDDPPDD", (0, 1): "DDDDPPDD", (0, 2): "DDDDPPDD",
    (1, 0): "DDDDDDDD", (1, 1): "DDDDDDDD", (1, 2): "DDDDDDDD",
}
# wrep evac engines per (layer, b): A/D ("AD" = halves in parallel)
WREPE_PAT = {0: "AA", 1: "AD"}
# qkT evac engines per layer (nt0, nt1)
QKTE_PAT = {0: "AA", 1: "AA"}
# watt load queue per lm: s=sync c=scalar g=gpsimd (DVE has no DGE port)
WATT_Q = "gcgcgc"

# urep load queue per (module, nt): 12 entries
UREP_Q = "ssssssssssss"
# ud store queue per (module, nt)
UD_Q = "ssssssssssss"
# out store queue per (m, dt): 12 entries (last module's last dt handled by fine path)
OUT_Q = {(0): "sscs", (1): "sscs", (2): "sscs"}
# fully fine-grained last module: per-(dt,nt) evac/mul engines + store queues
FINE_EVAC = "ADADADDA"
FINE_MUL = "DPDPDPPD"
FINE_STQ = "sgscsgcs"


def build_graph(nc):
    xt = nc.dram_tensor("xt", [MODS, D, BT], F16, kind="ExternalInput").ap()
    # host-repacked into SBUF layout for single-descriptor loads
    wqk = nc.dram_tensor("wqk", [P, LM, KC, P], F16, kind="ExternalInput").ap()
    bqk = nc.dram_tensor("bqk", [P, LM], F32, kind="ExternalInput").ap()
    watt = nc.dram_tensor("watt", [P, LM, KO, D], F8, kind="ExternalInput").ap()
    bout = nc.dram_tensor("bout", [P, LM, DT], F32, kind="ExternalInput").ap()
    out = nc.dram_tensor("out", [MODS, D, BT], F16, kind="ExternalOutput").ap()

    with tile.TileContext(nc) as tc, ExitStack() as ctx:
        const = ctx.enter_context(tc.tile_pool(name="const", bufs=1))
        xpool = ctx.enter_context(tc.tile_pool(name="xpool", bufs=16))
        xmp = ctx.enter_context(tc.tile_pool(name="xmp", bufs=2))
        wattp = ctx.enter_context(tc.tile_pool(name="wattp", bufs=6))
        qkp = ctx.enter_context(tc.tile_pool(name="qkp", bufs=3))
        uvp = ctx.enter_context(tc.tile_pool(name="uvp", bufs=4))
        natp = ctx.enter_context(tc.tile_pool(name="natp", bufs=3))
        mp = ctx.enter_context(tc.tile_pool(name="mp", bufs=8))
        pp_ = ctx.enter_context(tc.tile_pool(name="pp", bufs=6))
        wrp = ctx.enter_context(tc.tile_pool(name="wrp", bufs=3))
        urp = ctx.enter_context(tc.tile_pool(name="urp", bufs=3))
        zp = ctx.enter_context(tc.tile_pool(name="zp", bufs=10))
        rp = ctx.enter_context(tc.tile_pool(name="rp", bufs=3))
        op_ = ctx.enter_context(tc.tile_pool(name="op", bufs=3))
        dramp = ctx.enter_context(tc.tile_pool(name="dramp", bufs=4, space="DRAM"))

        # one PSUM pool, tag-partitioned: qk 2 banks, big 4 banks, sm 2 banks
        psp = ctx.enter_context(tc.tile_pool(name="psp", bufs=2, space="PSUM"))

        Q = {"s": nc.sync, "c": nc.scalar, "g": nc.gpsimd, "v": nc.vector}

        # identity first (Pool), head x/weight loads spread so the first
        # projection's chunks land earliest and nothing blocks ACT evacs
        ident = const.tile([P, P], F16)
        make_identity(nc, ident)
        identh = const.tile([P, 64], F16)
        make_identity(nc, identh[64:128, :])

        wqk_sb = const.tile([P, LM, KC, P], F16)
        bqk_sb = const.tile([P, LM], F32)
        bout_sb = const.tile([P, LM, DT], F32)
        nc.sync.dma_start(out=wqk_sb[:, 0, :, :], in_=wqk[:, 0])
        xt_cur = [[None] * KC for _ in range(MODS)]
        xv0 = xt[0].rearrange("(c p) bt -> p c bt", p=P)
        xm0_dmas = []
        for c in range(KC):
            xmc = xpool.tile([P, BT], F16, tag="x", name=f"xm0c{c}")
            d = (nc.sync if c % 2 == 0 else nc.gpsimd).dma_start(
                out=xmc, in_=xv0[:, c, :]
            )
            xm0_dmas.append(d)
            xt_cur[0][c] = xmc
        nc.sync.dma_start(out=bqk_sb, in_=bqk)
        # m1/m2 x tiles split in halves across queues; nosync deps keep the
        # scheduler from hoisting them ahead of module-0's critical chunks
        for m, q0, q1 in ((1, "c", "s"), (2, "g", "g")):
            xm = xmp.tile([P, KC, BT], F16, tag="xm", name=f"xm{m}")
            xv = xt[m].rearrange("(c p) bt -> p c bt", p=P)
            d0 = Q[q0].dma_start(out=xm[:, 0:2, :], in_=xv[:, 0:2, :])
            d1 = Q[q1].dma_start(out=xm[:, 2:KC, :], in_=xv[:, 2:KC, :])
            for d, dep in ((d0, xm0_dmas[1]), (d1, xm0_dmas[2 if m == 1 else 3])):
                add_dep_helper(d.ins, dep.ins, sync=False, reason="head order")
            for c in range(KC):
                xt_cur[m][c] = xm[:, c, :]
        wqk_rest = nc.sync.dma_start(
            out=wqk_sb[:, 1:MODS, :, :], in_=wqk[:, 1:MODS]
        )
        add_dep_helper(wqk_rest.ins, xm0_dmas[2].ins, sync=False,
                       reason="head order")


        # Per-layer state, keyed by layer index.
        S = {
            li: dict(Ms={}, Ps={}, uvTs=[None] * MODS, ut_dr=[None] * MODS,
                     watt_sb=[None] * MODS, wreps=[None] * MODS,
                     ureps=[None] * MODS, zTs=[None] * MODS)
            for li in range(L)
        }

        def s1(li, m):
            """Projections, u/v factors, Mqk forms for (layer, modality)."""
            st = S[li]
            lm = li * MODS + m
            qkT = qkp.tile([P, BT], F16, tag="qkT", name=f"qkT{lm}")
            uvT = uvp.tile([64, BT], F8, tag="uvT", name=f"uvT{lm}")
            ud = dramp.tile([4, NT, KO, 512], F8, tag="ut", name=f"ud{lm}")
            psqs = []
            # PE: proj(nt0) x4, proj(nt1) x4 emitted back-to-back so PE keeps
            # busy while ACT evacuates the first half.
            for nt in range(NT):
                sl = slice(nt * 512, (nt + 1) * 512)
                psq = psp.tile([P, 512], F32, tag="qk", bufs=2, name=f"psq{lm}{nt}")
                psqs.append(psq)
                for c in range(KC):
                    nc.tensor.matmul(
                        psq,
                        lhsT=wqk_sb[:, lm, c, :],
                        rhs=xt_cur[m][c][:, sl],
                        start=(c == 0),
                        stop=(c == KC - 1),
                    )
            psks = []
            last_evac = None
            for nt in range(NT):
                sl = slice(nt * 512, (nt + 1) * 512)
                if QKTE_PAT[li][nt] == "A":
                    last_evac = nc.scalar.activation(
                        out=qkT[:, sl], in_=psqs[nt], func=Ident,
                        scale=A[li], bias=bqk_sb[:, lm : lm + 1],
                    )
                else:
                    last_evac = nc.vector.tensor_scalar(
                        out=qkT[:, sl], in0=psqs[nt],
                        scalar1=A[li], scalar2=bqk_sb[:, lm : lm + 1],
                        op0=Mult, op1=Add,
                    )
                # shift k-rows to partitions 0:64 via identity matmul
                psk = psp.tile([64, 512], F32, tag="sm", bufs=2, name=f"psk{lm}{nt}")
                psks.append(psk)
                nc.tensor.matmul(
                    psk,
                    lhsT=identh[64:128, :],
                    rhs=qkT[64:128, sl],
                    tile_position=(64, 0),
                )
                nc.vector.tensor_mul(
                    out=uvT[:, sl], in0=qkT[0:64, sl], in1=psks[nt]
                )
                Q[UD_Q[lm * NT + nt]].dma_start(
                    out=ud[:, nt].rearrange("k c j -> c k j"), in_=uvT[0:32, sl]
                )
            st["uvTs"][m] = uvT
            st["ut_dr"][m] = ud

            # t-major u/v: transpose fp16 qkT chunks; evacuate only the k-half
            # to SBUF (ACT), then q(PSUM) * k(SBUF) in one DVE op
            pst = psp.tile([P, KO, P], F16, tag="sm", bufs=2, name=f"pst{lm}")
            for c8 in range(KO):
                nc.tensor.transpose(
                    pst[:, c8, :], qkT[:, c8 * P : (c8 + 1) * P], ident
                )
            t16 = natp.tile([P, KO, P], F16, tag="natt", name=f"natt{lm}")
            nc.vector.tensor_copy(out=t16, in_=pst)
            uv_nat = natp.tile([P, KO, 64], F8, tag="nat", name=f"nat{lm}")
            nc.gpsimd.tensor_mul(
                out=uv_nat, in0=t16[:, :, 0:64], in1=t16[:, :, 64:128]
            )

            forms = []
            if m in (0, 1):
                forms.append("L")
            if m in (1, 2):
                forms.append("R")
            slots = [(b, f) for b in range(BL) for f in forms]
            pm = psp.tile([32, len(slots), 32], F32, tag="sm", bufs=2, name=f"pm{lm}")
            for si, (b, f) in enumerate(slots):
                for cc in range(4):
                    ch = b * 4 + cc
                    if f == "L":
                        lhs = uv_nat[:, ch, 32:64]
                        rhs = uv_nat[:, ch, 0:32]
                    else:
                        lhs = uv_nat[:, ch, 0:32]
                        rhs = uv_nat[:, ch, 32:64]
                    nc.tensor.matmul(
                        pm[:, si, :], lhsT=lhs, rhs=rhs,
                        start=(cc == 0), stop=(cc == 3),
                    )
            msb = mp.tile([32, len(slots), 32], F8, tag="m", name=f"M{m}")
            nc.vector.tensor_scalar_mul(msb, pm, SM[li])
            for si, (b, f) in enumerate(slots):
                st["Ms"][(f, m, b)] = msb[:, si, :]
            wsb = wattp.tile([P, KO, D], F8, tag="watt", name=f"wsb{lm}")
            wdma = Q[WATT_Q[lm]].dma_start(out=wsb, in_=watt[:, lm])
            # scheduling-only edge: keep this bulky load from jumping ahead
            # of the module's own critical evacs on an idle queue
            add_dep_helper(wdma.ins, last_evac.ins, sync=False,
                           reason="pace watt load")
            st["watt_sb"][m] = wsb

        def pblock(li, js=range(MODS)):
            """Cross-modal P products; emits P tiled 4x along free dim so the
            w-matmul can write the partition-replicated wrep directly."""
            st = S[li]
            for j in js:
                for b in range(BL):
                    m1, m2 = [x for x in range(MODS) if x != j]
                    rhs4 = st["Ms"][("R", m2, b)][:, None, :].to_broadcast((32, 4, 32))
                    pps = psp.tile([64, 4, 32], F32, tag="sm", bufs=2,
                                   name=f"pps{li}{j}{b}")
                    nc.tensor.matmul(
                        pps[32:64],
                        lhsT=st["Ms"][("L", m1, b)],
                        rhs=rhs4,
                        tile_position=(0, 32),
                    )
                    ph = pp_.tile([64, 4, 32], F8, tag="p", name=f"ph{li}{j}{b}")
                    nc.vector.tensor_scalar_mul(ph[32:64], pps[32:64], TT[li])
                    st["Ps"][(j, b)] = ph

        def prep(li, m):
            """urep pair broadcast load + direct partition-replicated w (wrep)."""
            st = S[li]
            lm = li * MODS + m
            urall = urp.tile([P, NT, KO, 512], F8, tag="urep", name=f"ur{li}{m}")
            for nt in range(NT):
                src = st["ut_dr"][m][:, nt].rearrange("k c j -> k (c j)")[
                    :, None, :
                ].to_broadcast((4, 32, KO * 512))
                Q[UREP_Q[lm * NT + nt]].dma_start(
                    out=urall[:, nt].rearrange("p c j -> p (c j)"), in_=src
                )
            st["ureps"][m] = urall
            wrep = wrp.tile([P, BT], F8, tag="wrep", name=f"wrep{li}{m}")
            for b in range(BL):
                pw = psp.tile([P, 512], F32, tag="sm", bufs=2, name=f"pw{li}{m}{b}")
                nc.tensor.matmul(
                    pw,
                    lhsT=st["Ps"][(m, b)][32:64].rearrange("p a b -> p (a b)"),
                    rhs=st["uvTs"][m][32:64, b * 512 : (b + 1) * 512],
                    tile_position=(32, 0),
                )
                if WREPE_PAT[li][b] == "A":
                    wev = nc.scalar.activation(
                        out=wrep[:, b * 512 : (b + 1) * 512], in_=pw, func=Copy,
                        scale=CW[li],
                    )
                else:
                    wev = nc.vector.tensor_scalar_mul(
                        wrep[:, b * 512 : (b + 1) * 512], pw, CW[li]
                    )
            st["wreps"][m] = wrep
            return wev

        def zmuls(li, m):
            """Z.T pair tiles [128, 2, 512] = urep * wrep, split Pool/DVE."""
            st = S[li]
            urall = st["ureps"][m]
            zT = []
            for c2 in range(NPAIR):
                zc = zp.tile([P, 2, BT], F8, tag="zT", name=f"z{li}{m}c{c2}")
                zT.append(zc)
            # nt-major emission so big(nt0) can start while nt1 z-chunks build
            for idx in range(2 * NPAIR):
                b, c2 = idx // NPAIR, idx % NPAIR
                hs = slice(b * 512, (b + 1) * 512)
                eng = nc.gpsimd if ZMUL_PAT[(li, m)][idx] == "P" else nc.vector
                eng.tensor_mul(
                    out=zT[c2][:, :, hs],
                    in0=urall[:, b, 2 * c2 : 2 * c2 + 2, :],
                    in1=st["wreps"][m][:, None, hs].to_broadcast((P, 2, 512)),
                )
            st["zTs"][m] = zT

        def big(li, m):
            """DoubleRow fp8 matmul proj.T = Watt.T @ Z.T + residual combine.
            nt-granular psum tiles so evac overlaps accumulation."""
            st = S[li]
            lm = li * MODS + m
            zT = st["zTs"][m]
            if li == 0:
                xnew = [
                    xpool.tile([P, BT], F16, tag="x", name=f"xn{m}c{c}")
                    for c in range(KC)
                ]
            else:
                outm = out[m].rearrange("(t p) bt -> p t bt", p=P)
            for dt_i in range(DT):
                fine = li == 1 and m == 2
                res = rp.tile([P, BT], F16, tag="res")
                if li == 0:
                    tgt = xnew[dt_i]
                else:
                    tgt = op_.tile([P, BT], F16, tag="ost")
                for nt in range(NT):
                    sl = slice(nt * 512, (nt + 1) * 512)
                    pbig = psp.tile([P, 512], F32, tag="big", bufs=4, name="pbig")
                    for c2 in range(NPAIR):
                        nc.tensor.matmul(
                            pbig,
                            lhsT=st["watt_sb"][m][:, 2 * c2 : 2 * c2 + 2,
                                                  dt_i * P : (dt_i + 1) * P],
                            rhs=zT[c2][:, :, sl],
                            perf_mode=DR,
                            start=(c2 == 0),
                            stop=(c2 == NPAIR - 1),
                        )
                    hi = dt_i * NT + nt
                    ev = FINE_EVAC[hi] if fine else BIGE_PAT[(li, m)][hi]
                    if ev == "A":
                        nc.scalar.activation(
                            out=res[:, sl], in_=pbig, func=Ident,
                            scale=1.0 / KSC[li],
                            bias=bout_sb[:, lm, dt_i : dt_i + 1],
                        )
                    else:
                        nc.vector.tensor_scalar(
                            out=res[:, sl], in0=pbig,
                            scalar1=1.0 / KSC[li],
                            scalar2=bout_sb[:, lm, dt_i : dt_i + 1],
                            op0=Mult, op1=Add,
                        )
                    fm = FINE_MUL[hi] if fine else FMUL_PAT[(li, m)][hi]
                    eng = nc.vector if fm == "D" else nc.gpsimd
                    eng.tensor_mul(
                        out=tgt[:, sl], in0=res[:, sl],
                        in1=xt_cur[m][dt_i][:, sl],
                    )
                    if fine:
                        Q[FINE_STQ[hi]].dma_start(
                            out=outm[:, dt_i, sl], in_=tgt[:, sl]
                        )
                if li == 1 and not fine:
                    Q[OUT_Q[m][dt_i]].dma_start(out=outm[:, dt_i, :], in_=tgt)
            if li == 0:
                xt_cur[m] = xnew

        # ---- software-pipelined emission: layer-2 stage-1 hides under
        # ---- layer-1 big matmuls.
        for m in range(MODS):
            s1(0, m)
        pblock(0, js=(0,))
        wev00 = prep(0, 0)
        wq35 = nc.sync.dma_start(out=wqk_sb[:, MODS:, :, :], in_=wqk[:, MODS:])
        add_dep_helper(wq35.ins, wev00.ins, sync=False, reason="pace wqk l1")
        zmuls(0, 0)
        pblock(0, js=(1,))
        prep(0, 1)
        zmuls(0, 1)
        pblock(0, js=(2,))
        prep(0, 2)
        nc.gpsimd.dma_start(out=bout_sb, in_=bout)
        big(0, 0)
        s1(1, 0)
        zmuls(0, 2)
        big(0, 1)
        s1(1, 1)
        big(0, 2)
        s1(1, 2)
        pblock(1, js=(0,))
        prep(1, 0)
        zmuls(1, 0)
        pblock(1, js=(1,))
        prep(1, 1)
        zmuls(1, 1)
        pblock(1, js=(2,))
        prep(1, 2)
        big(1, 0)
        zmuls(1, 2)
        big(1, 1)
        big(1, 2)

    nc.finalize()
    return nc


_NC_CACHE = None


def _get_nc():
    global _NC_CACHE
    if _NC_CACHE is None:
        nc = bacc.Bacc("TRN2", target_bir_lowering=False, debug=False)
        _NC_CACHE = build_graph(nc)
    return _NC_CACHE


def make_in_maps(inputs):
    wqk = np.concatenate(
        [inputs["Wq1"], inputs["Wq2"], inputs["Wk1"], inputs["Wk2"]], axis=-1
    ).reshape(LM, D, 128)
    bqk_f = np.concatenate(
        [inputs["bq1"], inputs["bq2"], inputs["bk1"], inputs["bk2"]], axis=-1
    ).reshape(LM, 128).astype(np.float32)
    # pre-scale bias by the per-layer qkT scale (activation applies
    # out = in*scale + bias, so bias needs the same scale as the matmul)
    ascale = np.repeat([A[0], A[1]], MODS).astype(np.float32)[:, None]
    # repack to SBUF layouts: wqk [p, lm, c, w]; bqk [p, lm]; watt [p, lm, c, d];
    # bout [p, lm, dt]
    wqk_r = np.ascontiguousarray(
        wqk.reshape(LM, KC, P, 128).transpose(2, 0, 1, 3)
    ).astype(np.float16)
    bqk_r = np.ascontiguousarray((bqk_f * ascale).T)
    watt = np.asarray(inputs["Watt"], np.float32).reshape(LM, R * R, D)
    watt_f8 = np.clip(watt * SW, -240, 240).astype(ml_dtypes.float8_e4m3)
    watt_r = np.ascontiguousarray(
        watt_f8.reshape(LM, KO, P, D).transpose(2, 0, 1, 3)
    )
    bout = (np.asarray(inputs["batt"], np.float32) + np.float32(BETA)).reshape(
        LM, D
    )
    bout_r = np.ascontiguousarray(
        bout.reshape(LM, DT, P).transpose(2, 0, 1)
    )
    xs = [np.asarray(inputs[k], np.float32) for k in ("x_a", "x_t", "x_v")]
    in_maps = []
    for core in range(NCORES):
        sh = slice(core * BL, (core + 1) * BL)
        xts = np.stack(
            [np.ascontiguousarray(x[sh].reshape(BT, D).T) for x in xs]
        ).astype(np.float16)
        in_maps.append(
            {
                "xt": xts,
                "wqk": wqk_r,
                "bqk": bqk_r,
                "watt": watt_r,
                "bout": bout_r,
            }
        )
    return in_maps


def assemble(results):
    full = [np.empty((B, T, D), np.float32) for _ in range(MODS)]
    for core in range(NCORES):
        o = results[core]["out"]  # [MODS, D, BT] fp16
        for m in range(MODS):
            full[m][core * BL : (core + 1) * BL] = (
                o[m].T.reshape(BL, T, D).astype(np.float32)
            )
    return tuple(full)


def kernel(**inputs):
    nc = _get_nc()
    in_maps = make_in_maps(inputs)
    last_err = None
    for attempt in range(3):
        try:
            res = run_bass_kernel_spmd(nc, in_maps, core_ids=list(range(NCORES)))
            return assemble(res.results)
        except Exception as e:  # transient NRT_EXEC_UNIT_UNRECOVERABLE wedges
            last_err = e
            if attempt < 2:
                import time

                time.sleep(90)
    raise last_err


# revision 51
# speedup vs baseline: 1.0263x; 1.0140x over previous
"""Trainium2 Bass kernel for nn_AFW_63118839382657 (low-rank cross-modal bilinear net).

Key algebraic identity: G = (q1 outer q2) * (k1 outer k2) = (q1*k1) outer (q2*k2),
i.e. G is rank-1 per (b, t) with factors u = q1*k1, v = q2*k2 in R^32.
Everything then reduces to:
  Mqk[b]   = u[b].T @ v[b] / T                      (tiny matmuls)
  P_j[b]   = Mqk[m1] @ Mqk[m2]                      (32x32 matmuls)
  w[b]     = v[b] @ P_j[b]                          (w.T = P.T @ v.T)
  proj     = Z @ Watt,  Z[bt, k*32+o] = u[bt,k]*w[bt,o]
  out      = (proj + batt + beta) * X

fp8 version: the big matmul runs in float8e4 with MatmulPerfMode.DoubleRow.
All per-(b,t) factors and Watt are float8e4 with per-layer power-of-two scale
constants (A/SM/TT/CW/SW below).  X, projections, and outputs ride in fp16.

v2 scheduling: per-nt (512-token) PSUM tiles for projections and the big
matmul so evacuation overlaps accumulation; evac work split ACT/DVE (Pool has
no PSUM port); z-build split Pool/DVE; final muls split DVE/Pool; DMA traffic
spread across all five queues by load.  Data-parallel over batch: B=16 ->
2 per core across 8 NeuronCores, weights replicated, no collectives.
"""
import sys

for _p in ("/opt/trn_rl_repo", "/opt/pypackages"):
    if _p not in sys.path:
        sys.path.insert(0, _p)

import numpy as np
import ml_dtypes
from contextlib import ExitStack

import concourse.bass as bass
import concourse.mybir as mybir
import concourse.tile as tile
from concourse.tile import add_dep_helper
from concourse import bacc
from concourse.masks import make_identity
from concourse.bass_utils import run_bass_kernel_spmd

F32 = mybir.dt.float32
F16 = mybir.dt.float16
F8 = mybir.dt.float8e4
Copy = mybir.ActivationFunctionType.Copy
Ident = mybir.ActivationFunctionType.Identity
DR = mybir.MatmulPerfMode.DoubleRow
Mult = mybir.AluOpType.mult
Add = mybir.AluOpType.add

L, MODS, D, R, B, T = 2, 3, 512, 32, 16, 512
BETA = 0.1
NCORES = 8
BL = B // NCORES          # local batch = 2
BT = BL * T               # 1024
P = 128
KC = D // P               # 4 d-chunks (projection contraction)
KO = (R * R) // P         # 8 ko-chunks (big matmul contraction)
NPAIR = KO // 2           # 4 DoubleRow ko-chunk pairs
DT = D // P               # 4 d-tiles (big matmul output)
NT = BT // 512            # 2 bt-halves
LM = L * MODS

# per-layer fp8 scale constants (validated numerically: abs-max rel ~1.5e-3)
A = [2.0, 16.0]           # qkT scale
SM = [1.0 / 16, 1.0 / 16]  # Mqk evac scale (on top of a^4*T)
TT = [1.0 / 32, 1.0 / 16]  # P evac scale
CW = [1.0 / 16, 1.0 / 16]  # wrep evac scale
SW = 64.0                  # Watt scale
KSC = [A[li] ** 12 * T * T * SM[li] ** 2 * TT[li] * CW[li] * SW for li in range(L)]

# ---- engine assignment config (tuned against CoreSim) -----------------------
# zmul engines per (li, module): 8 ops (nt-major: b0c0..b0c3, b1c0..b1c3)
ZMUL_PAT = {
    (0, 0): "PDPPPPDP", (0, 1): "PDPPPPDP", (0, 2): "PDPPPPDP",
    (1, 0): "PPPPPDPP", (1, 1): "PPPPPDPP", (1, 2): "DPPDDPPD",
}
# big-evac engine per (li, m, (dt, nt)) half: A=ACT, D=DVE (8 chars)
BIGE_PAT = {
    (0, 0): "AAAADDAA", (0, 1): "AAAADDAA", (0, 2): "AAAADDAA",
    (1, 0): "AAAADAAA", (1, 1): "AAAADAAA", (1, 2): "AAAADAAA",
}
# final-mul engine per (li, m, (dt, nt)) half: D=DVE, P=Pool (8 chars)
FMUL_PAT = {
    (0, 0): "DDDDPPDD", (0, 1): "DDDDPPDD", (0, 2): "DDDDPPDD",
    (1, 0): "DDDDDDDD", (1, 1): "DDDDDDDD", (1, 2): "DDDDDDDD",
}
# wrep evac engines per (layer, b): A/D ("AD" = halves in parallel)
WREPE_PAT = {0: "AA", 1: "AD"}
# qkT evac engines per layer (nt0, nt1)
QKTE_PAT = {0: "AA", 1: "AA"}
# watt load queue per lm: s=sync c=scalar g=gpsimd (DVE has no DGE port)
WATT_Q = "gcgcgc"

# urep load queue per (module, nt): 12 entries
UREP_Q = "ssssssssssss"
# ud store queue per (module, nt)
UD_Q = "ssssssssssss"
# out store queue per (m, dt): 12 entries (last module's last dt handled by fine path)
OUT_Q = {(0): "sscs", (1): "sscs", (2): "sscs"}
# fully fine-grained last module: per-(dt,nt) evac/mul engines + store queues
FINE_EVAC = "ADADADDA"
FINE_MUL = "DPDPDPPD"
FINE_STQ = "sgscsgcs"


def build_graph(nc):
    xt = nc.dram_tensor("xt", [MODS, D, BT], F16, kind="ExternalInput").ap()
    # host-repacked into SBUF layout for single-descriptor loads
    wqk = nc.dram_tensor("wqk", [P, LM, KC, P], F16, kind="ExternalInput").ap()
    bqk = nc.dram_tensor("bqk", [P, LM], F32, kind="ExternalInput").ap()
    watt = nc.dram_tensor("watt", [P, LM, KO, D], F8, kind="ExternalInput").ap()
    bout = nc.dram_tensor("bout", [P, LM, DT], F32, kind="ExternalInput").ap()
    out = nc.dram_tensor("out", [MODS, D, BT], F16, kind="ExternalOutput").ap()

    with tile.TileContext(nc) as tc, ExitStack() as ctx:
        const = ctx.enter_context(tc.tile_pool(name="const", bufs=1))
        xpool = ctx.enter_context(tc.tile_pool(name="xpool", bufs=16))
        xmp = ctx.enter_context(tc.tile_pool(name="xmp", bufs=2))
        wattp = ctx.enter_context(tc.tile_pool(name="wattp", bufs=6))
        qkp = ctx.enter_context(tc.tile_pool(name="qkp", bufs=3))
        uvp = ctx.enter_context(tc.tile_pool(name="uvp", bufs=4))
        natp = ctx.enter_context(tc.tile_pool(name="natp", bufs=3))
        mp = ctx.enter_context(tc.tile_pool(name="mp", bufs=8))
        pp_ = ctx.enter_context(tc.tile_pool(name="pp", bufs=6))
        wrp = ctx.enter_context(tc.tile_pool(name="wrp", bufs=3))
        urp = ctx.enter_context(tc.tile_pool(name="urp", bufs=3))
        zp = ctx.enter_context(tc.tile_pool(name="zp", bufs=10))
        rp = ctx.enter_context(tc.tile_pool(name="rp", bufs=3))
        op_ = ctx.enter_context(tc.tile_pool(name="op", bufs=3))
        dramp = ctx.enter_context(tc.tile_pool(name="dramp", bufs=4, space="DRAM"))

        # one PSUM pool, tag-partitioned: qk 2 banks, big 4 banks, sm 2 banks
        psp = ctx.enter_context(tc.tile_pool(name="psp", bufs=2, space="PSUM"))

        Q = {"s": nc.sync, "c": nc.scalar, "g": nc.gpsimd, "v": nc.vector}

        # identity first (Pool), head x/weight loads spread so the first
        # projection's chunks land earliest and nothing blocks ACT evacs
        ident = const.tile([P, P], F16)
        make_identity(nc, ident)
        identh = const.tile([P, 64], F16)
        make_identity(nc, identh[64:128, :])

        wqk_sb = const.tile([P, LM, KC, P], F16)
        bqk_sb = const.tile([P, LM], F32)
        bout_sb = const.tile([P, LM, DT], F32)
        nc.sync.dma_start(out=wqk_sb[:, 0, :, :], in_=wqk[:, 0])
        xt_cur = [[None] * KC for _ in range(MODS)]
        xv0 = xt[0].rearrange("(c p) bt -> p c bt", p=P)
        xm0_dmas = []
        for c in range(KC):
            xmc = xpool.tile([P, BT], F16, tag="x", name=f"xm0c{c}")
            d = (nc.sync if c % 2 == 0 else nc.gpsimd).dma_start(
                out=xmc, in_=xv0[:, c, :]
            )
            xm0_dmas.append(d)
            xt_cur[0][c] = xmc
        nc.sync.dma_start(out=bqk_sb, in_=bqk)
        # m1/m2 x tiles split in halves across queues; nosync deps keep the
        # scheduler from hoisting them ahead of module-0's critical chunks
        for m, q0, q1 in ((1, "c", "s"), (2, "g", "g")):
            xm = xmp.tile([P, KC, BT], F16, tag="xm", name=f"xm{m}")
            xv = xt[m].rearrange("(c p) bt -> p c bt", p=P)
            d0 = Q[q0].dma_start(out=xm[:, 0:2, :], in_=xv[:, 0:2, :])
            d1 = Q[q1].dma_start(out=xm[:, 2:KC, :], in_=xv[:, 2:KC, :])
            for d, dep in ((d0, xm0_dmas[1]), (d1, xm0_dmas[2 if m == 1 else 3])):
                add_dep_helper(d.ins, dep.ins, sync=False, reason="head order")
            for c in range(KC):
                xt_cur[m][c] = xm[:, c, :]
        wqk_rest = nc.sync.dma_start(
            out=wqk_sb[:, 1:MODS, :, :], in_=wqk[:, 1:MODS]
        )
        add_dep_helper(wqk_rest.ins, xm0_dmas[2].ins, sync=False,
                       reason="head order")


        # Per-layer state, keyed by layer index.
        S = {
            li: dict(Ms={}, Ps={}, uvTs=[None] * MODS, ut_dr=[None] * MODS,
                     watt_sb=[None] * MODS, wreps=[None] * MODS,
                     ureps=[None] * MODS, zTs=[None] * MODS)
            for li in range(L)
        }

        def s1(li, m):
            """Projections, u/v factors, Mqk forms for (layer, modality)."""
            st = S[li]
            lm = li * MODS + m
            qkT = qkp.tile([P, BT], F16, tag="qkT", name=f"qkT{lm}")
            uvT = uvp.tile([64, BT], F8, tag="uvT", name=f"uvT{lm}")
            ud = dramp.tile([4, NT, KO, 512], F8, tag="ut", name=f"ud{lm}")
            psqs = []
            # PE: proj(nt0) x4, proj(nt1) x4 emitted back-to-back so PE keeps
            # busy while ACT evacuates the first half.
            for nt in range(NT):
                sl = slice(nt * 512, (nt + 1) * 512)
                psq = psp.tile([P, 512], F32, tag="qk", bufs=2, name=f"psq{lm}{nt}")
                psqs.append(psq)
                for c in range(KC):
                    nc.tensor.matmul(
                        psq,
                        lhsT=wqk_sb[:, lm, c, :],
                        rhs=xt_cur[m][c][:, sl],
                        start=(c == 0),
                        stop=(c == KC - 1),
                    )
            psks = []
            last_evac = None
            for nt in range(NT):
                sl = slice(nt * 512, (nt + 1) * 512)
                if QKTE_PAT[li][nt] == "A":
                    last_evac = nc.scalar.activation(
                        out=qkT[:, sl], in_=psqs[nt], func=Ident,
                        scale=A[li], bias=bqk_sb[:, lm : lm + 1],
                    )
                else:
                    last_evac = nc.vector.tensor_scalar(
                        out=qkT[:, sl], in0=psqs[nt],
                        scalar1=A[li], scalar2=bqk_sb[:, lm : lm + 1],
                        op0=Mult, op1=Add,
                    )
                # shift k-rows to partitions 0:64 via identity matmul
                psk = psp.tile([64, 512], F32, tag="sm", bufs=2, name=f"psk{lm}{nt}")
                psks.append(psk)
                nc.tensor.matmul(
                    psk,
                    lhsT=identh[64:128, :],
                    rhs=qkT[64:128, sl],
                    tile_position=(64, 0),
                )
                nc.vector.tensor_mul(
                    out=uvT[:, sl], in0=qkT[0:64, sl], in1=psks[nt]
                )
                Q[UD_Q[lm * NT + nt]].dma_start(
                    out=ud[:, nt].rearrange("k c j -> c k j"), in_=uvT[0:32, sl]
                )
            st["uvTs"][m] = uvT
            st["ut_dr"][m] = ud

            # t-major u/v: transpose fp16 qkT chunks; evacuate only the k-half
            # to SBUF (ACT), then q(PSUM) * k(SBUF) in one DVE op
            pst = psp.tile([P, KO, P], F16, tag="sm", bufs=2, name=f"pst{lm}")
            for c8 in range(KO):
                nc.tensor.transpose(
                    pst[:, c8, :], qkT[:, c8 * P : (c8 + 1) * P], ident
                )
            t16 = natp.tile([P, KO, P], F16, tag="natt", name=f"natt{lm}")
            nc.vector.tensor_copy(out=t16, in_=pst)
            uv_nat = natp.tile([P, KO, 64], F8, tag="nat", name=f"nat{lm}")
            nc.gpsimd.tensor_mul(
                out=uv_nat, in0=t16[:, :, 0:64], in1=t16[:, :, 64:128]
            )

            forms = []
            if m in (0, 1):
                forms.append("L")
            if m in (1, 2):
                forms.append("R")
            slots = [(b, f) for b in range(BL) for f in forms]
            pm = psp.tile([32, len(slots), 32], F32, tag="sm", bufs=2, name=f"pm{lm}")
            for si, (b, f) in enumerate(slots):
                for cc in range(4):
                    ch = b * 4 + cc
                    if f == "L":
                        lhs = uv_nat[:, ch, 32:64]
                        rhs = uv_nat[:, ch, 0:32]
                    else:
                        lhs = uv_nat[:, ch, 0:32]
                        rhs = uv_nat[:, ch, 32:64]
                    nc.tensor.matmul(
                        pm[:, si, :], lhsT=lhs, rhs=rhs,
                        start=(cc == 0), stop=(cc == 3),
                    )
            msb = mp.tile([32, len(slots), 32], F8, tag="m", name=f"M{m}")
            nc.vector.tensor_scalar_mul(msb, pm, SM[li])
            for si, (b, f) in enumerate(slots):
                st["Ms"][(f, m, b)] = msb[:, si, :]
            wsb = wattp.tile([P, KO, D], F8, tag="watt", name=f"wsb{lm}")
            wdma = Q[WATT_Q[lm]].dma_start(out=wsb, in_=watt[:, lm])
            # scheduling-only edge: keep this bulky load from jumping ahead
            # of the module's own critical evacs on an idle queue
            add_dep_helper(wdma.ins, last_evac.ins, sync=False,
                           reason="pace watt load")
            st["watt_sb"][m] = wsb

        def pblock(li, js=range(MODS)):
            """Cross-modal P products; emits P tiled 4x along free dim so the
            w-matmul can write the partition-replicated wrep directly."""
            st = S[li]
            for j in js:
                for b in range(BL):
                    m1, m2 = [x for x in range(MODS) if x != j]
                    rhs4 = st["Ms"][("R", m2, b)][:, None, :].to_broadcast((32, 4, 32))
                    pps = psp.tile([64, 4, 32], F32, tag="sm", bufs=2,
                                   name=f"pps{li}{j}{b}")
                    nc.tensor.matmul(
                        pps[32:64],
                        lhsT=st["Ms"][("L", m1, b)],
                        rhs=rhs4,
                        tile_position=(0, 32),
                    )
                    ph = pp_.tile([64, 4, 32], F8, tag="p", name=f"ph{li}{j}{b}")
                    nc.vector.tensor_scalar_mul(ph[32:64], pps[32:64], TT[li])
                    st["Ps"][(j, b)] = ph

        def prep(li, m):
            """urep pair broadcast load + direct partition-replicated w (wrep)."""
            st = S[li]
            lm = li * MODS + m
            urall = urp.tile([P, NT, KO, 512], F8, tag="urep", name=f"ur{li}{m}")
            for nt in range(NT):
                src = st["ut_dr"][m][:, nt].rearrange("k c j -> k (c j)")[
                    :, None, :
                ].to_broadcast((4, 32, KO * 512))
                Q[UREP_Q[lm * NT + nt]].dma_start(
                    out=urall[:, nt].rearrange("p c j -> p (c j)"), in_=src
                )
            st["ureps"][m] = urall
            wrep = wrp.tile([P, BT], F8, tag="wrep", name=f"wrep{li}{m}")
            for b in range(BL):
                pw = psp.tile([P, 512], F32, tag="sm", bufs=2, name=f"pw{li}{m}{b}")
                nc.tensor.matmul(
                    pw,
                    lhsT=st["Ps"][(m, b)][32:64].rearrange("p a b -> p (a b)"),
                    rhs=st["uvTs"][m][32:64, b * 512 : (b + 1) * 512],
                    tile_position=(32, 0),
                )
                if WREPE_PAT[li][b] == "A":
                    wev = nc.scalar.activation(
                        out=wrep[:, b * 512 : (b + 1) * 512], in_=pw, func=Copy,
                        scale=CW[li],
                    )
                else:
                    wev = nc.vector.tensor_scalar_mul(
                        wrep[:, b * 512 : (b + 1) * 512], pw, CW[li]
                    )
            st["wreps"][m] = wrep
            return wev

        def zmuls(li, m):
            """Z.T pair tiles [128, 2, 512] = urep * wrep, split Pool/DVE."""
            st = S[li]
            urall = st["ureps"][m]
            zT = []
            for c2 in range(NPAIR):
                zc = zp.tile([P, 2, BT], F8, tag="zT", name=f"z{li}{m}c{c2}")
                zT.append(zc)
            # nt-major emission so big(nt0) can start while nt1 z-chunks build
            for idx in range(2 * NPAIR):
                b, c2 = idx // NPAIR, idx % NPAIR
                hs = slice(b * 512, (b + 1) * 512)
                eng = nc.gpsimd if ZMUL_PAT[(li, m)][idx] == "P" else nc.vector
                eng.tensor_mul(
                    out=zT[c2][:, :, hs],
                    in0=urall[:, b, 2 * c2 : 2 * c2 + 2, :],
                    in1=st["wreps"][m][:, None, hs].to_broadcast((P, 2, 512)),
                )
            st["zTs"][m] = zT

        def big(li, m):
            """DoubleRow fp8 matmul proj.T = Watt.T @ Z.T + residual combine.
            nt-granular psum tiles so evac overlaps accumulation."""
            st = S[li]
            lm = li * MODS + m
            zT = st["zTs"][m]
            if li == 0:
                xnew = [
                    xpool.tile([P, BT], F16, tag="x", name=f"xn{m}c{c}")
                    for c in range(KC)
                ]
            else:
                outm = out[m].rearrange("(t p) bt -> p t bt", p=P)
            for dt_i in range(DT):
                fine = li == 1 and m == 2
                res = rp.tile([P, BT], F16, tag="res")
                if li == 0:
                    tgt = xnew[dt_i]
                else:
                    tgt = op_.tile([P, BT], F16, tag="ost")
                for nt in range(NT):
                    sl = slice(nt * 512, (nt + 1) * 512)
                    pbig = psp.tile([P, 512], F32, tag="big", bufs=4, name="pbig")
                    for c2 in range(NPAIR):
                        nc.tensor.matmul(
                            pbig,
                            lhsT=st["watt_sb"][m][:, 2 * c2 : 2 * c2 + 2,
                                                  dt_i * P : (dt_i + 1) * P],
                            rhs=zT[c2][:, :, sl],
                            perf_mode=DR,
                            start=(c2 == 0),
                            stop=(c2 == NPAIR - 1),
                        )
                    hi = dt_i * NT + nt
                    ev = FINE_EVAC[hi] if fine else BIGE_PAT[(li, m)][hi]
                    if ev == "A":
                        nc.scalar.activation(
                            out=res[:, sl], in_=pbig, func=Ident,
                            scale=1.0 / KSC[li],
                            bias=bout_sb[:, lm, dt_i : dt_i + 1],
                        )
                    else:
                        nc.vector.tensor_scalar(
                            out=res[:, sl], in0=pbig,
                            scalar1=1.0 / KSC[li],
                            scalar2=bout_sb[:, lm, dt_i : dt_i + 1],
                            op0=Mult, op1=Add,
                        )
                    fm = FINE_MUL[hi] if fine else FMUL_PAT[(li, m)][hi]
                    eng = nc.vector if fm == "D" else nc.gpsimd
                    eng.tensor_mul(
                        out=tgt[:, sl], in0=res[:, sl],
                        in1=xt_cur[m][dt_i][:, sl],
                    )
                    if fine:
                        Q[FINE_STQ[hi]].dma_start(
                            out=outm[:, dt_i, sl], in_=tgt[:, sl]
                        )
                if li == 1 and not fine:
                    Q[OUT_Q[m][dt_i]].dma_start(out=outm[:, dt_i, :], in_=tgt)
            if li == 0:
                xt_cur[m] = xnew

        # ---- software-pipelined emission: layer-2 stage-1 hides under
        # ---- layer-1 big matmuls.
        for m in range(MODS):
            s1(0, m)
        pblock(0, js=(0,))
        wev00 = prep(0, 0)
        wq35 = nc.sync.dma_start(out=wqk_sb[:, MODS:, :, :], in_=wqk[:, MODS:])
        add_dep_helper(wq35.ins, wev00.ins, sync=False, reason="pace wqk l1")
        zmuls(0, 0)
        pblock(0, js=(1,))
        prep(0, 1)
        zmuls(0, 1)
        pblock(0, js=(2,))
        prep(0, 2)
        nc.gpsimd.dma_start(out=bout_sb, in_=bout)
        big(0, 0)
        s1(1, 0)
        zmuls(0, 2)
        big(0, 1)
        s1(1, 1)
        big(0, 2)
        s1(1, 2)
        pblock(1, js=(0,))
        prep(1, 0)
        zmuls(1, 0)
        pblock(1, js=(1,))
        prep(1, 1)
        zmuls(1, 1)
        pblock(1, js=(2,))
        prep(1, 2)
        big(1, 0)
        zmuls(1, 2)
        big(1, 1)
        big(1, 2)

    nc.finalize()
    return nc


_NC_CACHE = None


def _get_nc():
    global _NC_CACHE
    if _NC_CACHE is None:
        nc = bacc.Bacc("TRN2", target_bir_lowering=False, debug=False)
        _NC_CACHE = build_graph(nc)
    return _NC_CACHE


def make_in_maps(inputs):
    wqk = np.concatenate(
        [inputs["Wq1"], inputs["Wq2"], inputs["Wk1"], inputs["Wk2"]], axis=-1
    ).reshape(LM, D, 128)
    bqk_f = np.concatenate(
        [inputs["bq1"], inputs["bq2"], inputs["bk1"], inputs["bk2"]], axis=-1
    ).reshape(LM, 128).astype(np.float32)
    # pre-scale bias by the per-layer qkT scale (activation applies
    # out = in*scale + bias, so bias needs the same scale as the matmul)
    ascale = np.repeat([A[0], A[1]], MODS).astype(np.float32)[:, None]
    # repack to SBUF layouts: wqk [p, lm, c, w]; bqk [p, lm]; watt [p, lm, c, d];
    # bout [p, lm, dt]
    wqk_r = np.ascontiguousarray(
        wqk.reshape(LM, KC, P, 128).transpose(2, 0, 1, 3)
    ).astype(np.float16)
    bqk_r = np.ascontiguousarray((bqk_f * ascale).T)
    watt = np.asarray(inputs["Watt"], np.float32).reshape(LM, R * R, D)
    watt_f8 = np.clip(watt * SW, -240, 240).astype(ml_dtypes.float8_e4m3)
    watt_r = np.ascontiguousarray(
        watt_f8.reshape(LM, KO, P, D).transpose(2, 0, 1, 3)
    )
    bout = (np.asarray(inputs["batt"], np.float32) + np.float32(BETA)).reshape(
        LM, D
    )
    bout_r = np.ascontiguousarray(
        bout.reshape(LM, DT, P).transpose(2, 0, 1)
    )
    xs = [np.asarray(inputs[k], np.float32) for k in ("x_a", "x_t", "x_v")]
    in_maps = []
    for core in range(NCORES):
        sh = slice(core * BL, (core + 1) * BL)
        xts = np.stack(
            [np.ascontiguousarray(x[sh].reshape(BT, D).T) for x in xs]
        ).astype(np.float16)
        in_maps.append(
            {
                "xt": xts,
                "wqk": wqk_r,
                "bqk": bqk_r,
                "watt": watt_r,
                "bout": bout_r,
            }
        )
    return in_maps


def assemble(results):
    full = [np.empty((B, T, D), np.float32) for _ in range(MODS)]
    for core in range(NCORES):
        o = results[core]["out"]  # [MODS, D, BT] fp16
        for m in range(MODS):
            full[m][core * BL : (core + 1) * BL] = (
                o[m].T.reshape(BL, T, D).astype(np.float32)
            )
    return tuple(full)


def kernel(**inputs):
    nc = _get_nc()
    in_maps = make_in_maps(inputs)
    last_err = None
    for attempt in range(3):
        try:
            res = run_bass_kernel_spmd(nc, in_maps, core_ids=list(range(NCORES)))
            return assemble(res.results)
        except Exception as e:  # transient NRT_EXEC_UNIT_UNRECOVERABLE wedges
            last_err = e
            if attempt < 2:
                import time

                time.sleep(90)
    raise last_err


# revision 53
# speedup vs baseline: 1.0628x; 1.0355x over previous
"""Trainium2 Bass kernel for nn_AFW_63118839382657 (low-rank cross-modal bilinear net).

Key algebraic identity: G = (q1 outer q2) * (k1 outer k2) = (q1*k1) outer (q2*k2),
i.e. G is rank-1 per (b, t) with factors u = q1*k1, v = q2*k2 in R^32.
Everything then reduces to:
  Mqk[b]   = u[b].T @ v[b] / T                      (tiny matmuls)
  P_j[b]   = Mqk[m1] @ Mqk[m2]                      (32x32 matmuls)
  w[b]     = v[b] @ P_j[b]                          (w.T = P.T @ v.T)
  proj     = Z @ Watt,  Z[bt, k*32+o] = u[bt,k]*w[bt,o]
  out      = (proj + batt + beta) * X

fp8 version: the big matmul runs in float8e4 with MatmulPerfMode.DoubleRow.
All per-(b,t) factors and Watt are float8e4 with per-layer power-of-two scale
constants (A/SM/TT/CW/SW below).  X, projections, and outputs ride in fp16.

v2 scheduling: per-nt (512-token) PSUM tiles for projections and the big
matmul so evacuation overlaps accumulation; evac work split ACT/DVE (Pool has
no PSUM port); z-build split Pool/DVE; final muls split DVE/Pool; DMA traffic
spread across all five queues by load.  Data-parallel over batch: B=16 ->
2 per core across 8 NeuronCores, weights replicated, no collectives.
"""
import sys

for _p in ("/opt/trn_rl_repo", "/opt/pypackages"):
    if _p not in sys.path:
        sys.path.insert(0, _p)

import numpy as np
import ml_dtypes
from contextlib import ExitStack

import concourse.bass as bass
import concourse.mybir as mybir
import concourse.tile as tile
from concourse.tile import add_dep_helper
from concourse import bacc
from concourse.masks import make_identity
from concourse.bass_utils import run_bass_kernel_spmd

F32 = mybir.dt.float32
F16 = mybir.dt.float16
F8 = mybir.dt.float8e4
Copy = mybir.ActivationFunctionType.Copy
Ident = mybir.ActivationFunctionType.Identity
DR = mybir.MatmulPerfMode.DoubleRow
Mult = mybir.AluOpType.mult
Add = mybir.AluOpType.add

L, MODS, D, R, B, T = 2, 3, 512, 32, 16, 512
BETA = 0.1
NCORES = 8
BL = B // NCORES          # local batch = 2
BT = BL * T               # 1024
P = 128
KC = D // P               # 4 d-chunks (projection contraction)
KO = (R * R) // P         # 8 ko-chunks (big matmul contraction)
NPAIR = KO // 2           # 4 DoubleRow ko-chunk pairs
DT = D // P               # 4 d-tiles (big matmul output)
NT = BT // 512            # 2 bt-halves
LM = L * MODS

# per-layer fp8 scale constants (validated numerically: abs-max rel ~1.5e-3)
A = [2.0, 16.0]           # qkT scale
SM = [1.0 / 16, 1.0 / 16]  # Mqk evac scale (on top of a^4*T)
TT = [1.0 / 32, 1.0 / 16]  # P evac scale
CW = [1.0 / 16, 1.0 / 16]  # wrep evac scale
SW = 64.0                  # Watt scale
KSC = [A[li] ** 12 * T * T * SM[li] ** 2 * TT[li] * CW[li] * SW for li in range(L)]

# ---- engine assignment config (tuned against CoreSim) -----------------------
# zmul engines per (li, module): 8 ops (nt-major: b0c0..b0c3, b1c0..b1c3)
ZMUL_PAT = {
    (0, 0): "PDPPDPPP", (0, 1): "PDPPDPPP", (0, 2): "PDPPDPPP",
    (1, 0): "PPPPPDPP", (1, 1): "PPPPPDPP", (1, 2): "DPDPDPPP",
}
# big-evac engine per (li, m, (dt, nt)) half: A=ACT, D=DVE (8 chars)
BIGE_PAT = {
    (0, 0): "AAAAADAA", (0, 1): "AAAAADAA", (0, 2): "AAAAADAA",
    (1, 0): "AAAADAAA", (1, 1): "AAAADAAA", (1, 2): "AAAADAAA",
}
# final-mul engine per (li, m, (dt, nt)) half: D=DVE, P=Pool (8 chars)
FMUL_PAT = {
    (0, 0): "DDDDPPDD", (0, 1): "DDDDPPDD", (0, 2): "DDDDPPDD",
    (1, 0): "DDDDDDDD", (1, 1): "DDDDDDDD", (1, 2): "DDDDDDDD",
}
# wrep evac engines per (layer, b): A/D ("AD" = halves in parallel)
WREPE_PAT = {0: "AA", 1: "AD"}
# qkT evac engines per layer (nt0, nt1)
QKTE_PAT = {0: "AA", 1: "AA"}
# watt load queue per lm: s=sync c=scalar g=gpsimd (DVE has no DGE port)
WATT_Q = "gcgcgc"

# urep load queue per (module, nt): 12 entries
UREP_Q = "ssssssssssss"
# ud store queue per (module, nt)
UD_Q = "ssssssssssss"
# out store queue per (m, dt): 12 entries (last module's last dt handled by fine path)
OUT_Q = {(0): "sscs", (1): "sscs", (2): "sscs"}
# fully fine-grained last module: per-(dt,nt) evac/mul engines + store queues
FINE_EVAC = "ADADADDA"
FINE_MUL = "DPDPDPPD"
FINE_STQ = "sgscsgcs"


def build_graph(nc):
    xt = nc.dram_tensor("xt", [MODS, D, BT], F16, kind="ExternalInput").ap()
    # host-repacked into SBUF layout for single-descriptor loads
    wqk = nc.dram_tensor("wqk", [P, LM, KC, P], F16, kind="ExternalInput").ap()
    bqk = nc.dram_tensor("bqk", [P, LM], F32, kind="ExternalInput").ap()
    watt = nc.dram_tensor("watt", [P, LM, KO, D], F8, kind="ExternalInput").ap()
    bout = nc.dram_tensor("bout", [P, LM, DT], F32, kind="ExternalInput").ap()
    out = nc.dram_tensor("out", [MODS, D, BT], F16, kind="ExternalOutput").ap()

    with tile.TileContext(nc) as tc, ExitStack() as ctx:
        const = ctx.enter_context(tc.tile_pool(name="const", bufs=1))
        xpool = ctx.enter_context(tc.tile_pool(name="xpool", bufs=16))
        xmp = ctx.enter_context(tc.tile_pool(name="xmp", bufs=2))
        wattp = ctx.enter_context(tc.tile_pool(name="wattp", bufs=6))
        qkp = ctx.enter_context(tc.tile_pool(name="qkp", bufs=3))
        uvp = ctx.enter_context(tc.tile_pool(name="uvp", bufs=4))
        natp = ctx.enter_context(tc.tile_pool(name="natp", bufs=3))
        mp = ctx.enter_context(tc.tile_pool(name="mp", bufs=8))
        pp_ = ctx.enter_context(tc.tile_pool(name="pp", bufs=6))
        wrp = ctx.enter_context(tc.tile_pool(name="wrp", bufs=3))
        urp = ctx.enter_context(tc.tile_pool(name="urp", bufs=3))
        zp = ctx.enter_context(tc.tile_pool(name="zp", bufs=10))
        rp = ctx.enter_context(tc.tile_pool(name="rp", bufs=3))
        op_ = ctx.enter_context(tc.tile_pool(name="op", bufs=3))
        dramp = ctx.enter_context(tc.tile_pool(name="dramp", bufs=4, space="DRAM"))

        # one PSUM pool, tag-partitioned: qk 2 banks, big 4 banks, sm 2 banks
        psp = ctx.enter_context(tc.tile_pool(name="psp", bufs=2, space="PSUM"))

        Q = {"s": nc.sync, "c": nc.scalar, "g": nc.gpsimd, "v": nc.vector}

        # identity first (Pool), head x/weight loads spread so the first
        # projection's chunks land earliest and nothing blocks ACT evacs
        ident = const.tile([P, P], F16)
        make_identity(nc, ident)
        identh = const.tile([P, 64], F16)
        make_identity(nc, identh[64:128, :])

        wqk_sb = const.tile([P, LM, KC, P], F16)
        bqk_sb = const.tile([P, LM], F32)
        bout_sb = const.tile([P, LM, DT], F32)
        nc.sync.dma_start(out=wqk_sb[:, 0, :, :], in_=wqk[:, 0])
        xt_cur = [[None] * KC for _ in range(MODS)]
        xv0 = xt[0].rearrange("(c p) bt -> p c bt", p=P)
        xm0_dmas = []
        for c in range(KC):
            xmc = xpool.tile([P, BT], F16, tag="x", name=f"xm0c{c}")
            d = (nc.sync if c % 2 == 0 else nc.gpsimd).dma_start(
                out=xmc, in_=xv0[:, c, :]
            )
            xm0_dmas.append(d)
            xt_cur[0][c] = xmc
        nc.sync.dma_start(out=bqk_sb, in_=bqk)
        # m1/m2 x tiles split in halves across queues; nosync deps keep the
        # scheduler from hoisting them ahead of module-0's critical chunks
        for m, q0, q1 in ((1, "c", "s"), (2, "g", "g")):
            xm = xmp.tile([P, KC, BT], F16, tag="xm", name=f"xm{m}")
            xv = xt[m].rearrange("(c p) bt -> p c bt", p=P)
            d0 = Q[q0].dma_start(out=xm[:, 0:2, :], in_=xv[:, 0:2, :])
            d1 = Q[q1].dma_start(out=xm[:, 2:KC, :], in_=xv[:, 2:KC, :])
            for d, dep in ((d0, xm0_dmas[1]), (d1, xm0_dmas[2 if m == 1 else 3])):
                add_dep_helper(d.ins, dep.ins, sync=False, reason="head order")
            for c in range(KC):
                xt_cur[m][c] = xm[:, c, :]
        wqk_rest = nc.sync.dma_start(
            out=wqk_sb[:, 1:MODS, :, :], in_=wqk[:, 1:MODS]
        )
        add_dep_helper(wqk_rest.ins, xm0_dmas[2].ins, sync=False,
                       reason="head order")


        # Per-layer state, keyed by layer index.
        S = {
            li: dict(Ms={}, Ps={}, uvTs=[None] * MODS, ut_dr=[None] * MODS,
                     watt_sb=[None] * MODS, wreps=[None] * MODS,
                     ureps=[None] * MODS, zTs=[None] * MODS)
            for li in range(L)
        }

        def s1(li, m):
            """Projections, u/v factors, Mqk forms for (layer, modality)."""
            st = S[li]
            lm = li * MODS + m
            qkT = qkp.tile([P, BT], F16, tag="qkT", name=f"qkT{lm}")
            uvT = uvp.tile([64, BT], F8, tag="uvT", name=f"uvT{lm}")
            ud = dramp.tile([4, NT, KO, 512], F8, tag="ut", name=f"ud{lm}")
            psqs = []
            # PE: proj(nt0) x4, proj(nt1) x4 emitted back-to-back so PE keeps
            # busy while ACT evacuates the first half.
            for nt in range(NT):
                sl = slice(nt * 512, (nt + 1) * 512)
                psq = psp.tile([P, 512], F32, tag="qk", bufs=2, name=f"psq{lm}{nt}")
                psqs.append(psq)
                for c in range(KC):
                    nc.tensor.matmul(
                        psq,
                        lhsT=wqk_sb[:, lm, c, :],
                        rhs=xt_cur[m][c][:, sl],
                        start=(c == 0),
                        stop=(c == KC - 1),
                    )
            psks = []
            last_evac = None
            for nt in range(NT):
                sl = slice(nt * 512, (nt + 1) * 512)
                if QKTE_PAT[li][nt] == "A":
                    last_evac = nc.scalar.activation(
                        out=qkT[:, sl], in_=psqs[nt], func=Ident,
                        scale=A[li], bias=bqk_sb[:, lm : lm + 1],
                    )
                else:
                    last_evac = nc.vector.tensor_scalar(
                        out=qkT[:, sl], in0=psqs[nt],
                        scalar1=A[li], scalar2=bqk_sb[:, lm : lm + 1],
                        op0=Mult, op1=Add,
                    )
                # shift k-rows to partitions 0:64 via identity matmul
                psk = psp.tile([64, 512], F32, tag="sm", bufs=2, name=f"psk{lm}{nt}")
                psks.append(psk)
                nc.tensor.matmul(
                    psk,
                    lhsT=identh[64:128, :],
                    rhs=qkT[64:128, sl],
                    tile_position=(64, 0),
                )
                nc.vector.tensor_mul(
                    out=uvT[:, sl], in0=qkT[0:64, sl], in1=psks[nt]
                )
                Q[UD_Q[lm * NT + nt]].dma_start(
                    out=ud[:, nt].rearrange("k c j -> c k j"), in_=uvT[0:32, sl]
                )
            st["uvTs"][m] = uvT
            st["ut_dr"][m] = ud

            # t-major u/v: transpose fp16 qkT chunks; evacuate only the k-half
            # to SBUF (ACT), then q(PSUM) * k(SBUF) in one DVE op
            pst = psp.tile([P, KO, P], F16, tag="sm", bufs=2, name=f"pst{lm}")
            for c8 in range(KO):
                nc.tensor.transpose(
                    pst[:, c8, :], qkT[:, c8 * P : (c8 + 1) * P], ident
                )
            t16 = natp.tile([P, KO, P], F16, tag="natt", name=f"natt{lm}")
            nc.vector.tensor_copy(out=t16, in_=pst)
            uv_nat = natp.tile([P, KO, 64], F8, tag="nat", name=f"nat{lm}")
            nc.gpsimd.tensor_mul(
                out=uv_nat, in0=t16[:, :, 0:64], in1=t16[:, :, 64:128]
            )

            forms = []
            if m in (0, 1):
                forms.append("L")
            if m in (1, 2):
                forms.append("R")
            slots = [(b, f) for b in range(BL) for f in forms]
            pm = psp.tile([32, len(slots), 32], F32, tag="sm", bufs=2, name=f"pm{lm}")
            for si, (b, f) in enumerate(slots):
                for cc in range(4):
                    ch = b * 4 + cc
                    if f == "L":
                        lhs = uv_nat[:, ch, 32:64]
                        rhs = uv_nat[:, ch, 0:32]
                    else:
                        lhs = uv_nat[:, ch, 0:32]
                        rhs = uv_nat[:, ch, 32:64]
                    nc.tensor.matmul(
                        pm[:, si, :], lhsT=lhs, rhs=rhs,
                        start=(cc == 0), stop=(cc == 3),
                    )
            msb = mp.tile([32, len(slots), 32], F8, tag="m", name=f"M{m}")
            nc.vector.tensor_scalar_mul(msb, pm, SM[li])
            for si, (b, f) in enumerate(slots):
                st["Ms"][(f, m, b)] = msb[:, si, :]
            wsb = wattp.tile([P, KO, D], F8, tag="watt", name=f"wsb{lm}")
            wdma = Q[WATT_Q[lm]].dma_start(out=wsb, in_=watt[:, lm])
            # scheduling-only edge: keep this bulky load from jumping ahead
            # of the module's own critical evacs on an idle queue
            add_dep_helper(wdma.ins, last_evac.ins, sync=False,
                           reason="pace watt load")
            st["watt_sb"][m] = wsb

        def pblock(li, js=range(MODS)):
            """Cross-modal P products; emits P tiled 4x along free dim so the
            w-matmul can write the partition-replicated wrep directly."""
            st = S[li]
            for j in js:
                for b in range(BL):
                    m1, m2 = [x for x in range(MODS) if x != j]
                    rhs4 = st["Ms"][("R", m2, b)][:, None, :].to_broadcast((32, 4, 32))
                    pps = psp.tile([64, 4, 32], F32, tag="sm", bufs=2,
                                   name=f"pps{li}{j}{b}")
                    nc.tensor.matmul(
                        pps[32:64],
                        lhsT=st["Ms"][("L", m1, b)],
                        rhs=rhs4,
                        tile_position=(0, 32),
                    )
                    ph = pp_.tile([64, 4, 32], F8, tag="p", name=f"ph{li}{j}{b}")
                    nc.vector.tensor_scalar_mul(ph[32:64], pps[32:64], TT[li])
                    st["Ps"][(j, b)] = ph

        def prep(li, m):
            """urep pair broadcast load + direct partition-replicated w (wrep)."""
            st = S[li]
            lm = li * MODS + m
            urall = urp.tile([P, NT, KO, 512], F8, tag="urep", name=f"ur{li}{m}")
            for nt in range(NT):
                src = st["ut_dr"][m][:, nt].rearrange("k c j -> k (c j)")[
                    :, None, :
                ].to_broadcast((4, 32, KO * 512))
                Q[UREP_Q[lm * NT + nt]].dma_start(
                    out=urall[:, nt].rearrange("p c j -> p (c j)"), in_=src
                )
            st["ureps"][m] = urall
            wrep = wrp.tile([P, BT], F8, tag="wrep", name=f"wrep{li}{m}")
            for b in range(BL):
                pw = psp.tile([P, 512], F32, tag="sm", bufs=2, name=f"pw{li}{m}{b}")
                nc.tensor.matmul(
                    pw,
                    lhsT=st["Ps"][(m, b)][32:64].rearrange("p a b -> p (a b)"),
                    rhs=st["uvTs"][m][32:64, b * 512 : (b + 1) * 512],
                    tile_position=(32, 0),
                )
                if WREPE_PAT[li][b] == "A":
                    wev = nc.scalar.activation(
                        out=wrep[:, b * 512 : (b + 1) * 512], in_=pw, func=Copy,
                        scale=CW[li],
                    )
                else:
                    wev = nc.vector.tensor_scalar_mul(
                        wrep[:, b * 512 : (b + 1) * 512], pw, CW[li]
                    )
            st["wreps"][m] = wrep
            return wev

        def zmuls(li, m):
            """Z.T pair tiles [128, 2, 512] = urep * wrep, split Pool/DVE."""
            st = S[li]
            urall = st["ureps"][m]
            zT = []
            for c2 in range(NPAIR):
                zc = zp.tile([P, 2, BT], F8, tag="zT", name=f"z{li}{m}c{c2}")
                zT.append(zc)
            # nt-major emission so big(nt0) can start while nt1 z-chunks build
            for idx in range(2 * NPAIR):
                b, c2 = idx // NPAIR, idx % NPAIR
                hs = slice(b * 512, (b + 1) * 512)
                eng = nc.gpsimd if ZMUL_PAT[(li, m)][idx] == "P" else nc.vector
                eng.tensor_mul(
                    out=zT[c2][:, :, hs],
                    in0=urall[:, b, 2 * c2 : 2 * c2 + 2, :],
                    in1=st["wreps"][m][:, None, hs].to_broadcast((P, 2, 512)),
                )
            st["zTs"][m] = zT

        def big(li, m):
            """DoubleRow fp8 matmul proj.T = Watt.T @ Z.T + residual combine.
            nt-granular psum tiles so evac overlaps accumulation."""
            st = S[li]
            lm = li * MODS + m
            zT = st["zTs"][m]
            if li == 0:
                xnew = [
                    xpool.tile([P, BT], F16, tag="x", name=f"xn{m}c{c}")
                    for c in range(KC)
                ]
            else:
                outm = out[m].rearrange("(t p) bt -> p t bt", p=P)
            for dt_i in range(DT):
                fine = li == 1 and m == 2
                res = rp.tile([P, BT], F16, tag="res")
                if li == 0:
                    tgt = xnew[dt_i]
                else:
                    tgt = op_.tile([P, BT], F16, tag="ost")
                for nt in range(NT):
                    sl = slice(nt * 512, (nt + 1) * 512)
                    pbig = psp.tile([P, 512], F32, tag="big", bufs=4, name="pbig")
                    for c2 in range(NPAIR):
                        nc.tensor.matmul(
                            pbig,
                            lhsT=st["watt_sb"][m][:, 2 * c2 : 2 * c2 + 2,
                                                  dt_i * P : (dt_i + 1) * P],
                            rhs=zT[c2][:, :, sl],
                            perf_mode=DR,
                            start=(c2 == 0),
                            stop=(c2 == NPAIR - 1),
                        )
                    hi = dt_i * NT + nt
                    ev = FINE_EVAC[hi] if fine else BIGE_PAT[(li, m)][hi]
                    if ev == "A":
                        nc.scalar.activation(
                            out=res[:, sl], in_=pbig, func=Ident,
                            scale=1.0 / KSC[li],
                            bias=bout_sb[:, lm, dt_i : dt_i + 1],
                        )
                    else:
                        nc.vector.tensor_scalar(
                            out=res[:, sl], in0=pbig,
                            scalar1=1.0 / KSC[li],
                            scalar2=bout_sb[:, lm, dt_i : dt_i + 1],
                            op0=Mult, op1=Add,
                        )
                    fm = FINE_MUL[hi] if fine else FMUL_PAT[(li, m)][hi]
                    eng = nc.vector if fm == "D" else nc.gpsimd
                    eng.tensor_mul(
                        out=tgt[:, sl], in0=res[:, sl],
                        in1=xt_cur[m][dt_i][:, sl],
                    )
                    if fine:
                        Q[FINE_STQ[hi]].dma_start(
                            out=outm[:, dt_i, sl], in_=tgt[:, sl]
                        )
                if li == 1 and not fine:
                    Q[OUT_Q[m][dt_i]].dma_start(out=outm[:, dt_i, :], in_=tgt)
            if li == 0:
                xt_cur[m] = xnew

        # ---- software-pipelined emission: layer-2 stage-1 hides under
        # ---- layer-1 big matmuls.
        for m in range(MODS):
            s1(0, m)
        pblock(0, js=(0,))
        wev00 = prep(0, 0)
        wq35 = nc.sync.dma_start(out=wqk_sb[:, MODS:, :, :], in_=wqk[:, MODS:])
        add_dep_helper(wq35.ins, wev00.ins, sync=False, reason="pace wqk l1")
        zmuls(0, 0)
        pblock(0, js=(1,))
        prep(0, 1)
        zmuls(0, 1)
        pblock(0, js=(2,))
        prep(0, 2)
        nc.gpsimd.dma_start(out=bout_sb, in_=bout)
        big(0, 0)
        s1(1, 0)
        zmuls(0, 2)
        big(0, 1)
        s1(1, 1)
        big(0, 2)
        s1(1, 2)
        pblock(1, js=(0,))
        prep(1, 0)
        zmuls(1, 0)
        pblock(1, js=(1,))
        prep(1, 1)
        zmuls(1, 1)
        pblock(1, js=(2,))
        prep(1, 2)
        big(1, 0)
        zmuls(1, 2)
        big(1, 1)
        big(1, 2)

    nc.finalize()
    return nc


_NC_CACHE = None


def _get_nc():
    global _NC_CACHE
    if _NC_CACHE is None:
        nc = bacc.Bacc("TRN2", target_bir_lowering=False, debug=False)
        _NC_CACHE = build_graph(nc)
    return _NC_CACHE


def make_in_maps(inputs):
    wqk = np.concatenate(
        [inputs["Wq1"], inputs["Wq2"], inputs["Wk1"], inputs["Wk2"]], axis=-1
    ).reshape(LM, D, 128)
    bqk_f = np.concatenate(
        [inputs["bq1"], inputs["bq2"], inputs["bk1"], inputs["bk2"]], axis=-1
    ).reshape(LM, 128).astype(np.float32)
    # pre-scale bias by the per-layer qkT scale (activation applies
    # out = in*scale + bias, so bias needs the same scale as the matmul)
    ascale = np.repeat([A[0], A[1]], MODS).astype(np.float32)[:, None]
    # repack to SBUF layouts: wqk [p, lm, c, w]; bqk [p, lm]; watt [p, lm, c, d];
    # bout [p, lm, dt]
    wqk_r = np.ascontiguousarray(
        wqk.reshape(LM, KC, P, 128).transpose(2, 0, 1, 3)
    ).astype(np.float16)
    bqk_r = np.ascontiguousarray((bqk_f * ascale).T)
    watt = np.asarray(inputs["Watt"], np.float32).reshape(LM, R * R, D)
    watt_f8 = np.clip(watt * SW, -240, 240).astype(ml_dtypes.float8_e4m3)
    watt_r = np.ascontiguousarray(
        watt_f8.reshape(LM, KO, P, D).transpose(2, 0, 1, 3)
    )
    bout = (np.asarray(inputs["batt"], np.float32) + np.float32(BETA)).reshape(
        LM, D
    )
    bout_r = np.ascontiguousarray(
        bout.reshape(LM, DT, P).transpose(2, 0, 1)
    )
    xs = [np.asarray(inputs[k], np.float32) for k in ("x_a", "x_t", "x_v")]
    in_maps = []
    for core in range(NCORES):
        sh = slice(core * BL, (core + 1) * BL)
        xts = np.stack(
            [np.ascontiguousarray(x[sh].reshape(BT, D).T) for x in xs]
        ).astype(np.float16)
        in_maps.append(
            {
                "xt": xts,
                "wqk": wqk_r,
                "bqk": bqk_r,
                "watt": watt_r,
                "bout": bout_r,
            }
        )
    return in_maps


def assemble(results):
    full = [np.empty((B, T, D), np.float32) for _ in range(MODS)]
    for core in range(NCORES):
        o = results[core]["out"]  # [MODS, D, BT] fp16
        for m in range(MODS):
            full[m][core * BL : (core + 1) * BL] = (
                o[m].T.reshape(BL, T, D).astype(np.float32)
            )
    return tuple(full)


def kernel(**inputs):
    nc = _get_nc()
    in_maps = make_in_maps(inputs)
    last_err = None
    for attempt in range(3):
        try:
            res = run_bass_kernel_spmd(nc, in_maps, core_ids=list(range(NCORES)))
            return assemble(res.results)
        except Exception as e:  # transient NRT_EXEC_UNIT_UNRECOVERABLE wedges
            last_err = e
            if attempt < 2:
                import time

                time.sleep(90)
    raise last_err
